# revision 2
# baseline (speedup 1.0000x reference)
"""3-layer GAT (graph attention network) on Trainium2 — Bass/Tile, 8-core SPMD.

Sharding: nodes are partitioned into 8 contiguous ranges (graph/data
parallel).  Each core owns the edges whose *destination* falls in its range.
Per layer:
  phase A : feat = x @ W and er = x @ (W@ar) for the core's own node slice
            (one PE matmul per 128-node block against [W | War]).  feat rows
            (bf16, 256 B) form the gather table; er rows go to a small local
            table.
  AllGather the 256 B/row feat table so every core can gather arbitrary
            src rows.
  edge    : per chunk (3 groups, ~6k edges) ONE bulk dma_gather per stream:
            er[dst] (local), feat[src] lo zone, feat[src] hi zone.  el is
            computed on device (feat*al, reduce over D); w =
            exp(leaky_relu(el+er)) via one fused vector op + scalar Exp;
            segment-sum of [w*feat | w] over destination nodes via PE
            matmuls with on-device-built one-hot matrices accumulated in
            PSUM (denominator folded into the same matmul); epilogue
            divides by the summed w (edge softmax), adds bias, applies relu.
Edge softmax skips the segment-max subtraction: alpha = exp(e)/sum(exp(e))
is mathematically identical and the logits here are O(1).

dma_gather uses int16 indices (max 32767), so edges are split per group into
a "lo" zone (src < 32768) and a "hi" zone (gathered from a base-offset view
of the table).  Edges are sorted by src within each (group, zone) for HBM
locality.  All indices are valid (pads point at row 0 and are killed by a
-1 dstrel -> all-zero one-hot column), so descriptor counts are
compile-time constants — one SPMD program serves all 8 cores; all
data-dependent structure lives in host-built index tables.
"""

import numpy as np

try:
    import ml_dtypes
    _BF16 = ml_dtypes.bfloat16
except ImportError:  # pragma: no cover
    _BF16 = None

# ---------------- static problem config (self-contained) ---------------------
N_CORES = 8
NEG_SLOPE = 0.2
P = 128
GROUP_E = 2048             # max edges per PSUM accumulation group
CHUNK_GROUPS = 3           # groups per gather chunk
SPLIT = 32768              # int16 index split point
# (in_dim, H, D, apply_relu) per layer
LAYERS = [(128, 4, 32, True), (128, 4, 32, True), (128, 1, 64, False)]
OUT_DIM = 64
ROW = 128                  # table row, bf16 elems (256 B = min gather elem)

_cache = {}
last_run_info = {}


# ============================ host-side preprocessing ========================

def _wrap16(vals, cols):
    """dma_gather index layout: entry i -> [i % 16, i // 16], replicated
    across the 8 groups of 16 partitions."""
    t = np.zeros((16, cols), np.int16)
    n = len(vals)
    t[np.arange(n) % 16, np.arange(n) // 16] = vals.astype(np.int16)
    return np.tile(t, (8, 1))


def _preprocess(src, dst, n_nodes, n_cores):
    npc = n_nodes // n_cores
    xj = (npc + P - 1) // P
    cores = []
    for c in range(n_cores):
        lo = c * npc
        m = (dst >= lo) & (dst < lo + npc)
        s = src[m].astype(np.int64)
        d = (dst[m] - lo).astype(np.int64)
        o = np.argsort(d, kind="stable")
        s, d = s[o], d[o]
        counts = np.bincount(d, minlength=npc)
        cum = np.zeros(npc + 1, np.int64)
        np.cumsum(counts, out=cum[1:])
        groups = []
        base = 0
        while base < npc:
            dmax = min(base + P, npc)
            limit = cum[base] + GROUP_E
            dend = int(np.searchsorted(cum, limit, side="right")) - 1
            dend = min(dend, dmax)
            if dend <= base:
                raise ValueError(f"dst {base} has degree > {GROUP_E}")
            groups.append((base, int(cum[base]), int(cum[dend])))
            base = dend
        cores.append((s, d, groups))

    ng = max(len(g) for (_, _, g) in cores)
    ng = ((ng + CHUNK_GROUPS - 1) // CHUNK_GROUPS) * CHUNK_GROUPS
    # per-group-index lo/hi tile counts (max across cores -> shared program)
    TL = np.zeros(ng, np.int64)
    TH = np.zeros(ng, np.int64)
    for (s, d, groups) in cores:
        for gi, (b, e0, e1) in enumerate(groups):
            nlo = int((s[e0:e1] < SPLIT).sum())
            nhi = (e1 - e0) - nlo
            TL[gi] = max(TL[gi], (nlo + P - 1) // P)
            TH[gi] = max(TH[gi], (nhi + P - 1) // P)
    lo_base = np.zeros(ng + 1, np.int64)   # tile offsets within lo zone
    hi_base = np.zeros(ng + 1, np.int64)
    np.cumsum(TL, out=lo_base[1:])
    np.cumsum(TH, out=hi_base[1:])
    SL = int(lo_base[ng]) * P              # lo-zone slots
    SH = int(hi_base[ng]) * P
    SLP = max(SL, 2048)                    # padded sizes for tensor shapes
    SHP = max(SH, 2048)
    CM = SL + SH                           # er slots (chunk-major layout)
    CMP = max(CM, 2048)

    per_core = []
    for c, (s, d, groups) in enumerate(cores):
        idx_lo = np.zeros(SL, np.int64)
        idx_hi = np.zeros(SH, np.int64)
        er_cm = np.zeros(CM, np.int64)
        dr_lo = np.full(SL, -1.0, np.float32)
        dr_hi = np.full(SH, -1.0, np.float32)
        for gi, (b, e0, e1) in enumerate(groups):
            eg_s = s[e0:e1]
            eg_d = d[e0:e1]
            lm = eg_s < SPLIT
            g0 = (gi // CHUNK_GROUPS) * CHUNK_GROUPS
            g1 = min(g0 + CHUNK_GROUPS, ng)
            cm0 = int(lo_base[g0] + hi_base[g0])      # tiles before chunk
            ltc = int(lo_base[g1] - lo_base[g0])      # chunk lo tiles
            for zone, msk in ((0, lm), (1, ~lm)):
                zs = eg_s[msk]
                zd = eg_d[msk]
                o2 = np.argsort(zs, kind="stable")    # src-sorted for HBM
                zs, zd = zs[o2], zd[o2]
                n = len(zs)
                if zone == 0:
                    o = int(lo_base[gi]) * P
                    idx_lo[o:o + n] = zs
                    dr_lo[o:o + n] = (zd - b).astype(np.float32)
                    cmo = (cm0 + int(lo_base[gi] - lo_base[g0])) * P
                else:
                    o = int(hi_base[gi]) * P
                    idx_hi[o:o + n] = zs - SPLIT
                    dr_hi[o:o + n] = (zd - b).astype(np.float32)
                    cmo = (cm0 + ltc + int(hi_base[gi] - hi_base[g0])) * P
                er_cm[cmo:cmo + n] = zd
        # scratch-row map (scratch row of own node n)
        srow = np.zeros(npc, np.int64)
        for gi, (b, e0, e1) in enumerate(groups):
            b_next = groups[gi + 1][0] if gi + 1 < len(groups) else npc
            srow[b:b_next] = gi * P + (np.arange(b, b_next) - b)
        nn = np.arange(xj * P)
        xi = np.zeros(xj * P, np.int64)
        valid = nn < npc
        xi[valid] = srow[nn[valid]]

        def _padcols(a, cols):
            out = np.zeros((a.shape[0], cols), a.dtype)
            out[:, :a.shape[1]] = a
            return out

        dl = dr_lo.reshape(SL // P, P).T.astype(_BF16) if SL else \
            np.zeros((P, 0), _BF16)
        dh = dr_hi.reshape(SH // P, P).T.astype(_BF16) if SH else \
            np.zeros((P, 0), _BF16)
        per_core.append(dict(
            idx_lo=_padcols(_wrap16(idx_lo, max(SL // 16, 1)), SLP // 16),
            idx_hi=_padcols(_wrap16(idx_hi, max(SH // 16, 1)), SHP // 16),
            idx_er=_padcols(_wrap16(er_cm, max(CM // 16, 1)), CMP // 16),
            dr_lo=_padcols(dl, SLP // P),
            dr_hi=_padcols(dh, SHP // P),
            x_idx=_wrap16(xi, (xj * P) // 16),
            srow=srow,
        ))
    meta = dict(ng=ng, TL=tuple(int(x) for x in TL),
                TH=tuple(int(x) for x in TH), SL=SL, SH=SH,
                SLP=SLP, SHP=SHP, CMP=CMP,
                xj=xj, npc=npc, n_nodes=n_nodes, n_cores=n_cores)
    return meta, per_core


# ============================ device program =================================

def _build_program(meta):
    import concourse.bass as bass
    import concourse.tile as tile
    from concourse import bacc, mybir

    def _midb(ap, n):
        # [P, D] -> [P, n, D] with the middle dim broadcast (step 0)
        return bass.AP(ap.tensor, ap.offset,
                       [list(ap.ap[0]), [0, n], list(ap.ap[1])])

    f32 = mybir.dt.float32
    bf16 = mybir.dt.bfloat16
    i16 = mybir.dt.int16
    AF = mybir.ActivationFunctionType
    OP = mybir.AluOpType

    ng, SL, SH = meta["ng"], meta["SL"], meta["SH"]
    SLP, SHP, CMP = meta["SLP"], meta["SHP"], meta["CMP"]
    TL, TH = meta["TL"], meta["TH"]
    xj, npc = meta["xj"], meta["npc"]
    n_nodes, n_cores = meta["n_nodes"], meta["n_cores"]
    NPCP = xj * P
    lo_base = np.concatenate([[0], np.cumsum(TL)]).astype(int)
    hi_base = np.concatenate([[0], np.cumsum(TH)]).astype(int)
    nchunk = ng // CHUNK_GROUPS

    nc = bacc.Bacc("TRN2", target_bir_lowering=False, debug=False,
                   enable_asserts=False, num_devices=n_cores,
                   num_swdge_queues=4)

    _qctr = [0]

    def _gather(out_ap3, in_ap, idxs2, ni, elem):
        q = _qctr[0] % 4
        nc.gpsimd.dma_gather(
            out_ap=out_ap3,
            in_ap=in_ap,
            idxs_ap=idxs2,
            num_idxs=ni,
            num_idxs_reg=ni,
            elem_size=elem,
            single_packet=False,
            queue_num=q,
        )
        _qctr[0] += 1

    t_feats = nc.dram_tensor("features_own", [NPCP, 128], f32,
                             kind="ExternalInput").ap()
    t_idx_lo = nc.dram_tensor("idx_lo", [P, SLP // 16], i16,
                              kind="ExternalInput").ap()
    t_idx_hi = nc.dram_tensor("idx_hi", [P, SHP // 16], i16,
                              kind="ExternalInput").ap()
    t_ier = nc.dram_tensor("idx_er", [P, CMP // 16], i16,
                           kind="ExternalInput").ap()
    t_dr_lo = nc.dram_tensor("dr_lo", [P, SLP // P], bf16,
                             kind="ExternalInput").ap()
    t_dr_hi = nc.dram_tensor("dr_hi", [P, SHP // P], bf16,
                             kind="ExternalInput").ap()
    t_x_idx = nc.dram_tensor("x_idx", [P, NPCP // 16], i16,
                             kind="ExternalInput").ap()
    t_iota = nc.dram_tensor("iota_rep", [P, P], bf16,
                            kind="ExternalInput").ap()
    t_ident = nc.dram_tensor("identity", [P, P], f32,
                             kind="ExternalInput").ap()
    t_WW, t_b, t_al = [], [], []
    for li, (ind, H, D, _) in enumerate(LAYERS):
        hd = H * D
        t_WW.append(nc.dram_tensor(f"WW{li}", [ind, hd + H], f32,
                                   kind="ExternalInput").ap())
        t_b.append(nc.dram_tensor(f"br{li}", [P, hd], f32,
                                  kind="ExternalInput").ap())
        t_al.append(nc.dram_tensor(f"albc{li}", [P, hd], bf16,
                                   kind="ExternalInput").ap())
    t_out = nc.dram_tensor("out", [ng * P, OUT_DIM], f32,
                           kind="ExternalOutput").ap()

    with tile.TileContext(nc) as tc:
        with (
            tc.tile_pool(name="const", bufs=1) as cpool,
            tc.tile_pool(name="big", bufs=1) as bigpool,
            tc.tile_pool(name="sb", bufs=3) as sb,
            tc.tile_pool(name="fg", bufs=2) as fgpool,
            tc.tile_pool(name="wp", bufs=2) as wpool,
            tc.tile_pool(name="ps", bufs=4, space="PSUM") as pspool,
            tc.tile_pool(name="psA", bufs=2, space="PSUM") as psA,
            tc.tile_pool(name="psB", bufs=2, space="PSUM") as psB,
            tc.tile_pool(name="dram", bufs=1, space="DRAM") as dram,
        ):
            # ---- constants ----
            ident = cpool.tile([P, P], f32)
            nc.sync.dma_start(ident[:], t_ident)
            iota = cpool.tile([P, P], bf16)
            nc.sync.dma_start(iota[:], t_iota)
            idx_lo = cpool.tile([P, SLP // 16], i16)
            nc.sync.dma_start(idx_lo[:], t_idx_lo)
            idx_hi = cpool.tile([P, SHP // 16], i16)
            nc.sync.dma_start(idx_hi[:], t_idx_hi)
            ier = cpool.tile([P, CMP // 16], i16)
            nc.sync.dma_start(ier[:], t_ier)
            dr_lo = cpool.tile([P, SLP // P], bf16)
            nc.sync.dma_start(dr_lo[:], t_dr_lo)
            dr_hi = cpool.tile([P, SHP // P], bf16)
            nc.sync.dma_start(dr_hi[:], t_dr_hi)
            xidx = cpool.tile([P, NPCP // 16], i16)
            nc.sync.dma_start(xidx[:], t_x_idx)
            WWs, Bs, ALs = [], [], []
            for li, (ind, H, D, _) in enumerate(LAYERS):
                hd = H * D
                w = cpool.tile([ind, hd + H], f32, tag=f"WW{li}")
                nc.sync.dma_start(w[:], t_WW[li])
                WWs.append(w)
                bb = cpool.tile([P, hd], f32, tag=f"br{li}")
                nc.sync.dma_start(bb[:], t_b[li])
                Bs.append(bb)
                aa = cpool.tile([P, hd], bf16, tag=f"albc{li}")
                nc.sync.dma_start(aa[:], t_al[li])
                ALs.append(aa)

            prev_scratch = None
            for li, (ind, H, D, apply_relu) in enumerate(LAYERS):
                hd = H * D
                # ---------------- phase A ----------------
                x_own = bigpool.tile([P, xj * ind], f32, tag="x_own")
                if li == 0:
                    nc.sync.dma_start(
                        x_own[:].rearrange("p (i d) -> p i d", d=ind),
                        t_feats.rearrange("(i p) d -> p i d", p=P))
                else:
                    _gather(x_own[:].rearrange("p (i d) -> p i d", d=ind),
                            prev_scratch[:], xidx[:], NPCP, ind)
                # table rows [feat bf16 | junk]
                tabsb = bigpool.tile([P, xj * ROW], bf16, tag="tabsb")
                er_own = sb.tile([P, xj * H], bf16, tag="er_own")
                for i in range(xj):
                    xT_ps = psA.tile([P, P], f32, tag="psA")
                    nc.tensor.transpose(
                        out=xT_ps[:], in_=x_own[:, i * ind:(i + 1) * ind],
                        identity=ident[:])
                    xT = sb.tile([P, ind], f32, tag="xT")
                    nc.any.tensor_copy(xT[:], xT_ps[:, :ind])
                    f_ps = psB.tile([P, hd + H], f32, tag="psB")
                    nc.tensor.matmul(out=f_ps[:], lhsT=xT[:],
                                     rhs=WWs[li][:], start=True, stop=True)
                    nc.any.tensor_copy(
                        tabsb[:, i * ROW:i * ROW + hd], f_ps[:, :hd])
                    nc.any.tensor_copy(er_own[:, i * H:(i + 1) * H],
                                       f_ps[:, hd:hd + H])

                tab_own_d = dram.tile([NPCP, ROW], bf16, tag=f"tab_own{li}")
                nc.sync.dma_start(
                    tab_own_d[:].rearrange("(i p) d -> p i d", p=P),
                    tabsb[:].rearrange("p (d2 d) -> p d2 d", d=ROW))
                er_own_d = dram.tile([NPCP, 128], bf16, tag=f"er_own{li}")
                nc.sync.dma_start(
                    er_own_d[:, :H].rearrange("(i p) h -> p i h", p=P),
                    er_own[:].rearrange("p (i h) -> p i h", h=H))

                # ---------------- all-gather ----------------
                tab_full = dram.tile([n_nodes, ROW], bf16,
                                     addr_space="Shared", tag=f"tab_full{li}")
                if n_cores == 1:
                    nc.sync.dma_start(tab_full[:], tab_own_d[:npc, :])
                else:
                    nc.gpsimd.collective_compute(
                        "AllGather", mybir.AluOpType.bypass,
                        replica_groups=[list(range(n_cores))],
                        ins=[tab_own_d[:npc, :]],
                        outs=[tab_full[:]],
                    )

                # ---------------- edge phase ----------------
                if li < 2:
                    scratch = dram.tile([ng * P, hd], f32, tag=f"scratch{li}")
                else:
                    scratch = None

                for k in range(nchunk):
                    g0 = k * CHUNK_GROUPS
                    g1 = g0 + CHUNK_GROUPS
                    lt0, lt1 = int(lo_base[g0]), int(lo_base[g1])
                    ht0, ht1 = int(hi_base[g0]), int(hi_base[g1])
                    ltc, htc = lt1 - lt0, ht1 - ht0
                    ct = ltc + htc
                    cm0 = lt0 + ht0
                    # one er gather for the whole chunk (lo slots | hi slots)
                    erg = fgpool.tile([P, max(ct, 1) * 128], bf16, tag="erg")
                    if ct:
                        _gather(erg[:].rearrange("p (j d) -> p j d", d=128),
                                er_own_d[:], ier[:, cm0 * 8:(cm0 + ct) * 8],
                                ct * P, 128)
                    zones = []
                    if ltc:
                        zones.append(("lo", lt0, lt1, 0, idx_lo, dr_lo,
                                      tab_full[:SPLIT, :]
                                      if n_nodes > SPLIT else tab_full[:]))
                    if htc:
                        zones.append(("hi", ht0, ht1, ltc, idx_hi, dr_hi,
                                      tab_full[SPLIT:, :]))
                    ztiles = {}
                    for (zn, tt0, tt1, eo, zidx, zdr, tab_ap) in zones:
                        ntile = tt1 - tt0
                        ni = ntile * P
                        fgt = fgpool.tile([P, ntile * ROW], bf16,
                                          tag=f"fg_{zn}")
                        _gather(fgt[:].rearrange("p (j d) -> p j d", d=ROW),
                                tab_ap, zidx[:, tt0 * 8:tt1 * 8], ni, ROW)
                        # el = sum_d feat*al  (on-device)
                        prod = wpool.tile([P, ntile * hd], bf16,
                                          tag=f"prod_{zn}")
                        nc.vector.tensor_tensor(
                            out=prod[:].rearrange("p (j d) -> p j d", d=hd),
                            in0=fgt[:].rearrange("p (j d) -> p j d",
                                                 d=ROW)[:, :, :hd],
                            in1=_midb(ALs[li][:], ntile),
                            op=OP.mult)
                        esum = wpool.tile([P, ntile * H], f32,
                                          tag=f"esum_{zn}")
                        nc.vector.tensor_reduce(
                            out=esum[:],
                            in_=prod[:].rearrange("p (j h d) -> p (j h) d",
                                                  h=H, d=D),
                            axis=mybir.AxisListType.X, op=OP.add)
                        # esum += er[dst]
                        nc.vector.tensor_tensor(
                            out=esum[:].rearrange("p (j h) -> p j h", h=H),
                            in0=esum[:].rearrange("p (j h) -> p j h", h=H),
                            in1=erg[:].rearrange("p (j d) -> p j d",
                                                 d=128)[:, eo:eo + ntile, :H],
                            op=OP.add)
                        # w = exp(max(esum, 0.2*esum))
                        lrl = wpool.tile([P, ntile * H], f32, tag=f"lrl_{zn}")
                        nc.vector.scalar_tensor_tensor(
                            out=lrl[:], in0=esum[:], scalar=NEG_SLOPE,
                            in1=esum[:], op0=OP.mult, op1=OP.max)
                        wch = wpool.tile([P, ntile * H], bf16, tag=f"w_{zn}")
                        nc.scalar.activation(wch[:], lrl[:], AF.Exp)
                        ztiles[zn] = (fgt, wch, tt0)

                    for g in range(g0, g1):
                        nt_tot = (int(lo_base[g + 1]) - int(lo_base[g]) +
                                  int(hi_base[g + 1]) - int(hi_base[g]))
                        dst_ap = (scratch[g * P:(g + 1) * P, :] if scratch
                                  is not None
                                  else t_out[g * P:(g + 1) * P, :])
                        if nt_tot == 0:
                            # group covers only empty dsts: out = (relu(b))
                            ot = sb.tile([P, hd], f32, tag="ot")
                            if apply_relu:
                                nc.vector.tensor_scalar_max(
                                    ot[:], Bs[li][:, :hd], 0.0)
                            else:
                                nc.vector.tensor_copy(ot[:], Bs[li][:, :hd])
                            nc.sync.dma_start(dst_ap, ot[:])
                            continue
                        ps = pspool.tile([P, hd + H], f32, tag="ps")
                        first = True
                        done = 0
                        for zn, zb0, zb1, zdr in (
                            ("lo", int(lo_base[g]), int(lo_base[g + 1]),
                             dr_lo),
                            ("hi", int(hi_base[g]), int(hi_base[g + 1]),
                             dr_hi),
                        ):
                            ntg = zb1 - zb0
                            if ntg == 0:
                                continue
                            fgt, wch, tt0 = ztiles[zn]
                            rel = zb0 - tt0
                            oh = sb.tile([P, ntg * P], bf16, tag="oh")
                            nc.vector.tensor_tensor(
                                out=oh[:].rearrange("p (t d) -> p t d", d=P),
                                in0=_midb(iota[:], ntg),
                                in1=zdr[:, zb0:zb1].to_broadcast([P, ntg, P]),
                                op=OP.is_equal)
                            mge = sb.tile([P, ntg * (hd + H)], bf16,
                                          tag="mge")
                            nc.vector.tensor_tensor(
                                out=mge[:].rearrange("p (t e) -> p t e",
                                                     e=hd + H)[:, :, :hd]
                                    .rearrange("p t (h d) -> p t h d", d=D),
                                in0=fgt[:].rearrange("p (t d) -> p t d",
                                                     d=ROW)[:, rel:rel + ntg,
                                                            :hd]
                                    .rearrange("p t (h d) -> p t h d", d=D),
                                in1=wch[:, rel * H:(rel + ntg) * H]
                                    .rearrange("p (t h) -> p t h", h=H)
                                    .to_broadcast([P, ntg, H, D]),
                                op=OP.mult)
                            nc.any.tensor_copy(
                                mge[:].rearrange("p (t e) -> p t e",
                                                 e=hd + H)[:, :, hd:],
                                wch[:, rel * H:(rel + ntg) * H]
                                    .rearrange("p (t h) -> p t h", h=H))
                            for t in range(ntg):
                                done += 1
                                nc.tensor.matmul(
                                    out=ps[:],
                                    lhsT=oh[:, t * P:(t + 1) * P],
                                    rhs=mge[:, t * (hd + H):
                                            (t + 1) * (hd + H)],
                                    start=first, stop=(done == nt_tot))
                                first = False
                        # epilogue: divide, bias, relu
                        den = sb.tile([P, H], f32, tag="den")
                        nc.vector.tensor_scalar_max(den[:], ps[:, hd:hd + H],
                                                    1e-12)
                        rec = sb.tile([P, H], f32, tag="rec")
                        nc.vector.reciprocal(rec[:], den[:])
                        ot = sb.tile([P, hd], f32, tag="ot")
                        nc.vector.tensor_tensor(
                            out=ot[:].rearrange("p (h d) -> p h d", d=D),
                            in0=ps[:, :hd].rearrange("p (h d) -> p h d", d=D),
                            in1=rec[:].to_broadcast([P, H, D]),
                            op=OP.mult)
                        nc.vector.tensor_tensor(out=ot[:], in0=ot[:],
                                                in1=Bs[li][:, :hd], op=OP.add)
                        if apply_relu:
                            nc.vector.tensor_scalar_max(ot[:], ot[:], 0.0)
                        nc.sync.dma_start(dst_ap, ot[:])
                prev_scratch = scratch
    nc.compile()
    return nc


# ============================ entry point ====================================

def _meta_key(meta):
    return (meta["ng"], meta["TL"], meta["TH"], meta["SL"], meta["SH"],
            meta["xj"], meta["npc"], meta["n_nodes"], meta["n_cores"])


def _get_compiled(meta):
    key = _meta_key(meta)
    if key not in _cache:
        _cache[key] = _build_program(meta)
    return _cache[key]


def _make_in_maps(inputs, meta, per_core):
    f32 = np.float32
    xj, npc = meta["xj"], meta["npc"]
    n_cores = meta["n_cores"]
    iota_rep = np.tile(np.arange(P, dtype=f32).astype(_BF16), (P, 1))
    ident = np.eye(P, dtype=f32)
    common = {"iota_rep": iota_rep, "identity": ident}
    for li in range(len(LAYERS)):
        ind, H, D, _ = LAYERS[li]
        W = np.asarray(inputs[f"W{li}"], f32)
        al = np.asarray(inputs[f"al{li}"], f32)
        ar = np.asarray(inputs[f"ar{li}"], f32)
        b = np.asarray(inputs[f"b{li}"], f32)
        hd = H * D
        ar_flat = np.zeros((hd, H), f32)
        for h in range(H):
            ar_flat[h * D:(h + 1) * D, h] = ar[h]
        WW = np.concatenate([W, (W @ ar_flat).astype(f32)], axis=1)
        common[f"WW{li}"] = np.ascontiguousarray(WW)
        common[f"br{li}"] = np.tile(b[None, :], (P, 1)).astype(f32)
        common[f"albc{li}"] = np.tile(al.reshape(1, hd), (P, 1)).astype(_BF16)

    feats = np.asarray(inputs["features"], f32)
    in_maps = []
    for c in range(n_cores):
        pc = per_core[c]
        fo = np.zeros((xj * P, 128), f32)
        fo[:npc] = feats[c * npc:(c + 1) * npc]
        in_maps.append({
            **common,
            "features_own": fo,
            "idx_lo": pc["idx_lo"], "idx_hi": pc["idx_hi"],
            "idx_er": pc["idx_er"],
            "dr_lo": pc["dr_lo"], "dr_hi": pc["dr_hi"],
            "x_idx": pc["x_idx"],
        })
    return in_maps


def kernel(**inputs):
    from concourse import bass_utils

    src = np.asarray(inputs["src"]).astype(np.int64)
    dst = np.asarray(inputs["dst"]).astype(np.int64)
    n_nodes = np.asarray(inputs["features"]).shape[0]
    meta, per_core = _preprocess(src, dst, n_nodes, N_CORES)
    nc = _get_compiled(meta)
    in_maps = _make_in_maps(inputs, meta, per_core)
    n_cores = meta["n_cores"]
    res = bass_utils.run_bass_kernel_spmd(
        nc, in_maps, core_ids=list(range(n_cores)),
        trace=bool(last_run_info.get("trace", False)))
    last_run_info["exec_time_ns"] = res.exec_time_ns
    last_run_info["profile_json"] = res.profile_json
    last_run_info["res"] = res

    npc = meta["npc"]
    out = np.empty((n_nodes, OUT_DIM), np.float32)
    for c in range(n_cores):
        rows = per_core[c]["srow"]
        out[c * npc:(c + 1) * npc] = res.results[c]["out"][rows]
    return out


# revision 5
# speedup vs baseline: 1.6062x; 1.6062x over previous
"""3-layer GAT (graph attention network) on Trainium2 — Bass/Tile, 8-core SPMD.

Sharding: nodes are partitioned into 8 contiguous ranges (graph/data
parallel).  Each core owns the edges whose *destination* falls in its range.

All per-core node data lives in "slot" order: destination nodes are packed
into ng groups of 128 slots (group g covers a window of <=128 consecutive
nodes); slot = g*128 + (node - group_base).  The host permutes the input
features into slot order, the epilogue writes outputs in slot order (so the
next layer's phase A needs NO gather), and per-group attention tables live
at static slot addresses (shared SPMD program across cores).

Per layer:
  phase A : one PE matmul per 128-slot block against [W | Wal | War] gives
            feat, el, er.  Table rows [feat|el] (bf16, 512 B) are written to
            DRAM; er (8 B/slot) to a packed local table.
  AllGather the table so every core can gather arbitrary src rows.
  edge    : per chunk, feat[src] rows are dma_gathered with indices split
            over all 4 SWDGE queues (desc-gen runs on a DSP pair per queue).
            One-hot matrices (host-built fp8, streamed from DRAM — both
            normal and transposed layouts) drive PE matmuls:
              er_edge = ohT.T @ er_blk            (per tile, [128,H])
              [msg | den] = oh.T @ [w*feat | w]   (accumulated per group)
            w = exp(leaky_relu(el+er)) via one fused DVE op + scalar Exp.
            Epilogue divides by den (edge softmax), adds bias, relu.
Edge softmax skips the segment-max subtraction: alpha = exp(e)/sum(exp(e))
is mathematically identical and the logits here are O(1).

dma_gather uses int16 indices (max 32767), so edges are split per group into
a "lo" zone (src slot < 32768) and a "hi" zone (gathered from a base-offset
view of the table).  Edges are sorted by src within each (group, zone) for
HBM locality.  All indices are valid (pads point at row 0; pad one-hot
columns are all-zero), so descriptor counts are compile-time constants.
"""

import numpy as np

try:
    import ml_dtypes
    _BF16 = ml_dtypes.bfloat16
    _FP8 = ml_dtypes.float8_e4m3
except ImportError:  # pragma: no cover
    _BF16 = None
    _FP8 = None

# ---------------- static problem config (self-contained) ---------------------
N_CORES = 8
NEG_SLOPE = 0.2
P = 128
GROUP_E = 2048             # max edges per PSUM accumulation group
CHUNK_GROUPS = 2           # groups per gather chunk
SPLIT = 32768              # int16 index split point
# (in_dim, H, D, apply_relu) per layer
LAYERS = [(128, 4, 32, True), (128, 4, 32, True), (128, 1, 64, False)]
OUT_DIM = 64
ROW = 256                  # table row, bf16 elems (512 B): [feat | el | pad]
HMAX = 4                   # er table row width (elems)

_cache = {}
last_run_info = {}


# ============================ host-side preprocessing ========================

def _wrap16(vals, cols):
    """dma_gather index layout: entry i -> [i % 16, i // 16], replicated
    across the 8 groups of 16 partitions."""
    t = np.zeros((16, cols), np.int16)
    n = len(vals)
    t[np.arange(n) % 16, np.arange(n) // 16] = vals.astype(np.int16)
    return np.tile(t, (8, 1))


def _preprocess(src, dst, n_nodes, n_cores):
    npc = n_nodes // n_cores
    cores = []
    for c in range(n_cores):
        lo = c * npc
        m = (dst >= lo) & (dst < lo + npc)
        s = src[m].astype(np.int64)
        d = (dst[m] - lo).astype(np.int64)
        o = np.argsort(d, kind="stable")
        s, d = s[o], d[o]
        counts = np.bincount(d, minlength=npc)
        cum = np.zeros(npc + 1, np.int64)
        np.cumsum(counts, out=cum[1:])
        groups = []
        base = 0
        while base < npc:
            dmax = min(base + P, npc)
            limit = cum[base] + GROUP_E
            dend = int(np.searchsorted(cum, limit, side="right")) - 1
            dend = min(dend, dmax)
            if dend <= base:
                raise ValueError(f"dst {base} has degree > {GROUP_E}")
            groups.append((base, int(cum[base]), int(cum[dend])))
            base = dend
        cores.append((s, d, groups))

    ng = max(len(g) for (_, _, g) in cores)
    ng = ((ng + CHUNK_GROUPS - 1) // CHUNK_GROUPS) * CHUNK_GROUPS
    SLOTS = ng * P                       # per-core slot rows
    # node -> slot map per core (slot = gi*128 + node - base), -1 for none
    node_slot = []
    for c, (s, d, groups) in enumerate(cores):
        srow = np.zeros(npc, np.int64)
        for gi, (b, e0, e1) in enumerate(groups):
            b_next = groups[gi + 1][0] if gi + 1 < len(groups) else npc
            srow[b:b_next] = gi * P + (np.arange(b, b_next) - b)
        node_slot.append(srow)
    # global slot of node n
    gslot = np.empty(n_nodes, np.int64)
    for c in range(n_cores):
        gslot[c * npc:(c + 1) * npc] = c * SLOTS + node_slot[c]

    # per-group lo/hi tile counts (max across cores -> shared program)
    TL = np.zeros(ng, np.int64)
    TH = np.zeros(ng, np.int64)
    for (s, d, groups) in cores:
        gs = gslot[s]                    # src as global slot
        for gi, (b, e0, e1) in enumerate(groups):
            nlo = int((gs[e0:e1] < SPLIT).sum())
            nhi = (e1 - e0) - nlo
            TL[gi] = max(TL[gi], (nlo + P - 1) // P)
            TH[gi] = max(TH[gi], (nhi + P - 1) // P)
    lo_base = np.zeros(ng + 1, np.int64)
    hi_base = np.zeros(ng + 1, np.int64)
    np.cumsum(TL, out=lo_base[1:])
    np.cumsum(TH, out=hi_base[1:])
    SL = int(lo_base[ng]) * P
    SH = int(hi_base[ng]) * P
    SLP = max(SL, 2048)
    SHP = max(SH, 2048)
    CT = SL // P + SH // P               # total tiles, chunk-major layout

    per_core = []
    for c, (s, d, groups) in enumerate(cores):
        gs = gslot[s]
        idx_lo = np.zeros(SL, np.int64)
        idx_hi = np.zeros(SH, np.int64)
        # chunk-major dstrel per slot (-1 = pad) for host one-hot build
        dr_cm = np.full(CT * P, -1.0, np.float32)
        for gi, (b, e0, e1) in enumerate(groups):
            eg_s = gs[e0:e1]
            eg_d = d[e0:e1]
            lm = eg_s < SPLIT
            g0 = (gi // CHUNK_GROUPS) * CHUNK_GROUPS
            g1 = min(g0 + CHUNK_GROUPS, ng)
            cm0 = int(lo_base[g0] + hi_base[g0])
            ltc = int(lo_base[g1] - lo_base[g0])
            for zone, msk in ((0, lm), (1, ~lm)):
                zs = eg_s[msk]
                zd = eg_d[msk]
                o2 = np.argsort(zs, kind="stable")    # src-sorted for HBM
                zs, zd = zs[o2], zd[o2]
                n = len(zs)
                if zone == 0:
                    o = int(lo_base[gi]) * P
                    idx_lo[o:o + n] = zs
                    cmo = (cm0 + int(lo_base[gi] - lo_base[g0])) * P
                else:
                    o = int(hi_base[gi]) * P
                    idx_hi[o:o + n] = zs - SPLIT
                    cmo = (cm0 + ltc + int(hi_base[gi] - hi_base[g0])) * P
                dr_cm[cmo:cmo + n] = (zd - b).astype(np.float32)

        # one-hot (fp8) in chunk-major tile order; oh: [e%128, tile, j]
        # layout [P, CT*128]; ohT: [j, tile, e] layout [P, CT*128]
        drt = dr_cm.reshape(CT, P)                    # [tile, e]
        eye = (drt[:, :, None] == np.arange(P, dtype=np.float32)[None, None, :])
        oh = eye.astype(_FP8)                         # [tile, e, j]
        oh_d = np.ascontiguousarray(
            oh.transpose(1, 0, 2).reshape(P, CT * P))  # [e, tile*j]
        ohT_d = np.ascontiguousarray(
            oh.transpose(2, 0, 1).reshape(P, CT * P))  # [j, tile*e]

        def _padcols(a, cols):
            out = np.zeros((a.shape[0], cols), a.dtype)
            out[:, :a.shape[1]] = a
            return out

        per_core.append(dict(
            idx_lo=_padcols(_wrap16(idx_lo, max(SL // 16, 1)), SLP // 16),
            idx_hi=_padcols(_wrap16(idx_hi, max(SH // 16, 1)), SHP // 16),
            oh=oh_d, ohT=ohT_d,
            srow=node_slot[c],
        ))
    meta = dict(ng=ng, TL=tuple(int(x) for x in TL),
                TH=tuple(int(x) for x in TH), SL=SL, SH=SH,
                SLP=SLP, SHP=SHP, CT=CT,
                npc=npc, n_nodes=n_nodes, n_cores=n_cores)
    return meta, per_core


# ============================ device program =================================

def _build_program(meta):
    import concourse.bass as bass
    import concourse.tile as tile
    from concourse import bacc, mybir

    f32 = mybir.dt.float32
    bf16 = mybir.dt.bfloat16
    fp8 = mybir.dt.float8e4
    i16 = mybir.dt.int16
    AF = mybir.ActivationFunctionType
    OP = mybir.AluOpType

    ng, SL, SH = meta["ng"], meta["SL"], meta["SH"]
    SLP, SHP, CT = meta["SLP"], meta["SHP"], meta["CT"]
    TL, TH = meta["TL"], meta["TH"]
    npc = meta["npc"]
    n_cores = meta["n_cores"]
    SLOTS = ng * P
    NTOT = SLOTS * n_cores               # all-gathered table rows
    lo_base = np.concatenate([[0], np.cumsum(TL)]).astype(int)
    hi_base = np.concatenate([[0], np.cumsum(TH)]).astype(int)
    nchunk = ng // CHUNK_GROUPS

    nc = bacc.Bacc("TRN2", target_bir_lowering=False, debug=False,
                   enable_asserts=False, num_devices=n_cores,
                   num_swdge_queues=4)

    def _gather4(out_ap3, in_ap, idxs2, t0, t1, elem):
        """gather tiles [t0,t1) of a zone, striped over the 4 queues."""
        nt = t1 - t0
        step = (nt + 3) // 4
        q = 0
        for j0 in range(0, nt, step):
            j1 = min(j0 + step, nt)
            nc.gpsimd.dma_gather(
                out_ap=out_ap3[:, j0:j1, :],
                in_ap=in_ap,
                idxs_ap=idxs2[:, (t0 + j0) * 8:(t0 + j1) * 8],
                num_idxs=(j1 - j0) * P,
                num_idxs_reg=(j1 - j0) * P,
                elem_size=elem,
                single_packet=False,
                queue_num=q,
            )
            q += 1

    t_feats = nc.dram_tensor("features_own", [SLOTS, 128], f32,
                             kind="ExternalInput").ap()
    t_idx_lo = nc.dram_tensor("idx_lo", [P, SLP // 16], i16,
                              kind="ExternalInput").ap()
    t_idx_hi = nc.dram_tensor("idx_hi", [P, SHP // 16], i16,
                              kind="ExternalInput").ap()
    t_oh = nc.dram_tensor("oh", [P, CT * P], fp8, kind="ExternalInput").ap()
    t_ohT = nc.dram_tensor("ohT", [P, CT * P], fp8, kind="ExternalInput").ap()
    t_ident = nc.dram_tensor("identity", [P, P], f32,
                             kind="ExternalInput").ap()
    t_WW, t_b = [], []
    for li, (ind, H, D, _) in enumerate(LAYERS):
        hd = H * D
        t_WW.append(nc.dram_tensor(f"WW{li}", [ind, hd + 2 * H], f32,
                                   kind="ExternalInput").ap())
        t_b.append(nc.dram_tensor(f"br{li}", [P, hd], f32,
                                  kind="ExternalInput").ap())
    t_out = nc.dram_tensor("out", [SLOTS, OUT_DIM], f32,
                           kind="ExternalOutput").ap()

    with tile.TileContext(nc) as tc:
        with (
            tc.tile_pool(name="const", bufs=1) as cpool,
            tc.tile_pool(name="big", bufs=1) as bigpool,
            tc.tile_pool(name="sb", bufs=3) as sb,
            tc.tile_pool(name="fg", bufs=2) as fgpool,
            tc.tile_pool(name="wp", bufs=3) as wpool,
            tc.tile_pool(name="ps", bufs=3, space="PSUM") as pspool,
            tc.tile_pool(name="per", bufs=2, space="PSUM") as perpool,
            tc.tile_pool(name="psA", bufs=2, space="PSUM") as psA,
            tc.tile_pool(name="psB", bufs=1, space="PSUM") as psB,
            tc.tile_pool(name="dram", bufs=1, space="DRAM") as dram,
        ):
            # ---- constants ----
            ident = cpool.tile([P, P], f32)
            nc.sync.dma_start(ident[:], t_ident)
            idx_lo = cpool.tile([P, SLP // 16], i16)
            nc.sync.dma_start(idx_lo[:], t_idx_lo)
            idx_hi = cpool.tile([P, SHP // 16], i16)
            nc.sync.dma_start(idx_hi[:], t_idx_hi)
            WWs, Bs = [], []
            for li, (ind, H, D, _) in enumerate(LAYERS):
                hd = H * D
                w = cpool.tile([ind, hd + 2 * H], f32, tag=f"WW{li}")
                nc.sync.dma_start(w[:], t_WW[li])
                WWs.append(w)
                bb = cpool.tile([P, hd], f32, tag=f"br{li}")
                nc.sync.dma_start(bb[:], t_b[li])
                Bs.append(bb)

            prev_scratch = None
            for li, (ind, H, D, apply_relu) in enumerate(LAYERS):
                hd = H * D
                # ---------------- phase A ----------------
                x_own = bigpool.tile([P, ng * ind], f32, tag="x_own")
                if li == 0:
                    nc.sync.dma_start(
                        x_own[:].rearrange("p (i d) -> p i d", d=ind),
                        t_feats.rearrange("(i p) d -> p i d", p=P))
                else:
                    nc.sync.dma_start(
                        x_own[:].rearrange("p (i d) -> p i d", d=ind),
                        prev_scratch[:].rearrange("(i p) d -> p i d", p=P))
                # table rows [feat | el | junk]
                tabsb = bigpool.tile([P, ng * ROW], bf16, tag="tabsb")
                er_own = sb.tile([P, ng * HMAX], bf16, tag="er_own")
                for i in range(ng):
                    xT_ps = psA.tile([P, P], f32, tag="psA")
                    nc.tensor.transpose(
                        out=xT_ps[:], in_=x_own[:, i * ind:(i + 1) * ind],
                        identity=ident[:])
                    xT = sb.tile([P, ind], f32, tag="xT")
                    nc.any.tensor_copy(xT[:], xT_ps[:, :ind])
                    f_ps = psB.tile([P, hd + 2 * H], f32, tag="psB")
                    nc.tensor.matmul(out=f_ps[:], lhsT=xT[:],
                                     rhs=WWs[li][:], start=True, stop=True)
                    nc.any.tensor_copy(
                        tabsb[:, i * ROW:i * ROW + hd + H],
                        f_ps[:, :hd + H])
                    nc.any.tensor_copy(er_own[:, i * HMAX:i * HMAX + H],
                                       f_ps[:, hd + H:hd + 2 * H])

                tab_own_d = dram.tile([SLOTS, ROW], bf16, tag=f"tab_own{li}")
                nc.sync.dma_start(
                    tab_own_d[:].rearrange("(i p) d -> p i d", p=P),
                    tabsb[:].rearrange("p (d2 d) -> p d2 d", d=ROW))
                er_own_d = dram.tile([SLOTS, HMAX], bf16, tag=f"er_own{li}")
                nc.sync.dma_start(
                    er_own_d[:].rearrange("(i p) h -> p i h", p=P),
                    er_own[:].rearrange("p (i h) -> p i h", h=HMAX))

                # ---------------- all-gather ----------------
                tab_full = dram.tile([NTOT, ROW], bf16,
                                     addr_space="Shared", tag=f"tab_full{li}")
                if n_cores == 1:
                    nc.sync.dma_start(tab_full[:], tab_own_d[:])
                else:
                    nc.gpsimd.collective_compute(
                        "AllGather", mybir.AluOpType.bypass,
                        replica_groups=[list(range(n_cores))],
                        ins=[tab_own_d[:]],
                        outs=[tab_full[:]],
                    )

                # ---------------- edge phase ----------------
                if li < 2:
                    scratch = dram.tile([SLOTS, hd], f32, tag=f"scratch{li}")
                else:
                    scratch = None

                for k in range(nchunk):
                    g0 = k * CHUNK_GROUPS
                    g1 = g0 + CHUNK_GROUPS
                    lt0, lt1 = int(lo_base[g0]), int(lo_base[g1])
                    ht0, ht1 = int(hi_base[g0]), int(hi_base[g1])
                    ltc, htc = lt1 - lt0, ht1 - ht0
                    ct = ltc + htc
                    cm0 = lt0 + ht0
                    if ct == 0:
                        for g in range(g0, g1):
                            dst_ap = (scratch[g * P:(g + 1) * P, :]
                                      if scratch is not None
                                      else t_out[g * P:(g + 1) * P, :])
                            ot = sb.tile([P, hd], f32, tag="ot")
                            if apply_relu:
                                nc.vector.tensor_scalar_max(
                                    ot[:], Bs[li][:, :hd], 0.0)
                            else:
                                nc.vector.tensor_copy(ot[:], Bs[li][:, :hd])
                            nc.sync.dma_start(dst_ap, ot[:])
                        continue
                    # one-hot (normal + transposed) streamed from DRAM
                    oh = fgpool.tile([P, ct * P], fp8, tag="oh")
                    nc.sync.dma_start(oh[:], t_oh[:, cm0 * P:(cm0 + ct) * P])
                    ohT = fgpool.tile([P, ct * P], fp8, tag="ohT")
                    nc.sync.dma_start(ohT[:],
                                      t_ohT[:, cm0 * P:(cm0 + ct) * P])
                    # feat gathers, striped over the 4 queues
                    zones = {}
                    if ltc:
                        fgt = fgpool.tile([P, ltc * ROW], bf16, tag="fg_lo")
                        _gather4(fgt[:].rearrange("p (j d) -> p j d", d=ROW),
                                 tab_full[:SPLIT, :], idx_lo, lt0, lt1, ROW)
                        zones["lo"] = fgt
                    if htc:
                        fgt = fgpool.tile([P, htc * ROW], bf16, tag="fg_hi")
                        _gather4(fgt[:].rearrange("p (j d) -> p j d", d=ROW),
                                 tab_full[SPLIT:, :], idx_hi, ht0, ht1, ROW)
                        zones["hi"] = fgt

                    for g in range(g0, g1):
                        gl0, gl1 = int(lo_base[g]), int(lo_base[g + 1])
                        gh0, gh1 = int(hi_base[g]), int(hi_base[g + 1])
                        nt_tot = (gl1 - gl0) + (gh1 - gh0)
                        dst_ap = (scratch[g * P:(g + 1) * P, :]
                                  if scratch is not None
                                  else t_out[g * P:(g + 1) * P, :])
                        if nt_tot == 0:
                            ot = sb.tile([P, hd], f32, tag="ot")
                            if apply_relu:
                                nc.vector.tensor_scalar_max(
                                    ot[:], Bs[li][:, :hd], 0.0)
                            else:
                                nc.vector.tensor_copy(ot[:], Bs[li][:, :hd])
                            nc.sync.dma_start(dst_ap, ot[:])
                            continue
                        # er for this group's 128 slots (static address!)
                        er_blk = sb.tile([P, HMAX], bf16, tag="er_blk")
                        nc.sync.dma_start(er_blk[:],
                                          er_own_d[g * P:(g + 1) * P, :])
                        # chunk-tile offset of this group's zones
                        zinfo = []
                        if gl1 > gl0:
                            zinfo.append(("lo", gl0 - lt0, gl1 - gl0,
                                          gl0 - lt0))
                        if gh1 > gh0:
                            zinfo.append(("hi", ltc + (gh0 - ht0),
                                          gh1 - gh0, gh0 - ht0))
                        # er_edge per tile via PE: ohT_t.T @ er_blk
                        er_ps = perpool.tile([P, nt_tot * H], f32, tag="er")
                        ti = 0
                        for (zn, ctile, ntg, rel) in zinfo:
                            for t in range(ntg):
                                nc.tensor.matmul(
                                    out=er_ps[:, (ti + t) * H:
                                              (ti + t + 1) * H],
                                    lhsT=ohT[:, (ctile + t) * P:
                                             (ctile + t + 1) * P],
                                    rhs=er_blk[:, :H],
                                    start=True, stop=True)
                            ti += ntg
                        # esum = el + er ; w = exp(lrelu)
                        esum = wpool.tile([P, nt_tot * H], f32, tag="esum")
                        ti = 0
                        for (zn, ctile, ntg, rel) in zinfo:
                            nc.vector.tensor_tensor(
                                out=esum[:, ti * H:(ti + ntg) * H]
                                    .rearrange("p (t h) -> p t h", h=H),
                                in0=zones[zn][:]
                                    .rearrange("p (t d) -> p t d",
                                               d=ROW)[:, rel:rel + ntg,
                                                      hd:hd + H],
                                in1=er_ps[:, ti * H:(ti + ntg) * H]
                                    .rearrange("p (t h) -> p t h", h=H),
                                op=OP.add)
                            ti += ntg
                        lrl = wpool.tile([P, nt_tot * H], f32, tag="lrl")
                        nc.vector.scalar_tensor_tensor(
                            out=lrl[:], in0=esum[:], scalar=NEG_SLOPE,
                            in1=esum[:], op0=OP.mult, op1=OP.max)
                        wch = wpool.tile([P, nt_tot * H], bf16, tag="w")
                        nc.scalar.activation(wch[:], lrl[:], AF.Exp)
                        # mge = [w*feat | w]
                        mge = sb.tile([P, nt_tot * (hd + H)], bf16, tag="mge")
                        ti = 0
                        for (zn, ctile, ntg, rel) in zinfo:
                            nc.vector.tensor_tensor(
                                out=mge[:, ti * (hd + H):
                                        (ti + ntg) * (hd + H)]
                                    .rearrange("p (t e) -> p t e",
                                               e=hd + H)[:, :, :hd]
                                    .rearrange("p t (h d) -> p t h d", d=D),
                                in0=zones[zn][:]
                                    .rearrange("p (t d) -> p t d",
                                               d=ROW)[:, rel:rel + ntg, :hd]
                                    .rearrange("p t (h d) -> p t h d", d=D),
                                in1=wch[:, ti * H:(ti + ntg) * H]
                                    .rearrange("p (t h) -> p t h", h=H)
                                    .to_broadcast([P, ntg, H, D]),
                                op=OP.mult)
                            ti += ntg
                        nc.any.tensor_copy(
                            mge[:].rearrange("p (t e) -> p t e",
                                             e=hd + H)[:, :, hd:],
                            wch[:].rearrange("p (t h) -> p t h", h=H))
                        # segment matmuls, denominator folded
                        ps = pspool.tile([P, hd + H], f32, tag="ps")
                        ti = 0
                        first = True
                        for (zn, ctile, ntg, rel) in zinfo:
                            for t in range(ntg):
                                nc.tensor.matmul(
                                    out=ps[:],
                                    lhsT=oh[:, (ctile + t) * P:
                                            (ctile + t + 1) * P],
                                    rhs=mge[:, (ti + t) * (hd + H):
                                            (ti + t + 1) * (hd + H)],
                                    start=first,
                                    stop=(ti + t + 1 == nt_tot))
                                first = False
                            ti += ntg
                        # epilogue: divide, bias, relu
                        den = sb.tile([P, H], f32, tag="den")
                        nc.vector.tensor_scalar_max(den[:], ps[:, hd:hd + H],
                                                    1e-12)
                        rec = sb.tile([P, H], f32, tag="rec")
                        nc.vector.reciprocal(rec[:], den[:])
                        ot = sb.tile([P, hd], f32, tag="ot")
                        nc.vector.tensor_tensor(
                            out=ot[:].rearrange("p (h d) -> p h d", d=D),
                            in0=ps[:, :hd].rearrange("p (h d) -> p h d", d=D),
                            in1=rec[:].to_broadcast([P, H, D]),
                            op=OP.mult)
                        nc.vector.tensor_tensor(out=ot[:], in0=ot[:],
                                                in1=Bs[li][:, :hd], op=OP.add)
                        if apply_relu:
                            nc.vector.tensor_scalar_max(ot[:], ot[:], 0.0)
                        nc.sync.dma_start(dst_ap, ot[:])
                prev_scratch = scratch
    nc.compile()
    return nc


# ============================ entry point ====================================

def _meta_key(meta):
    return (meta["ng"], meta["TL"], meta["TH"], meta["SL"], meta["SH"],
            meta["npc"], meta["n_nodes"], meta["n_cores"])


def _get_compiled(meta):
    key = _meta_key(meta)
    if key not in _cache:
        _cache[key] = _build_program(meta)
    return _cache[key]


def _make_in_maps(inputs, meta, per_core):
    f32 = np.float32
    npc = meta["npc"]
    ng = meta["ng"]
    n_cores = meta["n_cores"]
    ident = np.eye(P, dtype=f32)
    common = {"identity": ident}
    for li in range(len(LAYERS)):
        ind, H, D, _ = LAYERS[li]
        W = np.asarray(inputs[f"W{li}"], f32)
        al = np.asarray(inputs[f"al{li}"], f32)
        ar = np.asarray(inputs[f"ar{li}"], f32)
        b = np.asarray(inputs[f"b{li}"], f32)
        hd = H * D
        al_flat = np.zeros((hd, H), f32)
        ar_flat = np.zeros((hd, H), f32)
        for h in range(H):
            al_flat[h * D:(h + 1) * D, h] = al[h]
            ar_flat[h * D:(h + 1) * D, h] = ar[h]
        WW = np.concatenate([W, (W @ al_flat).astype(f32),
                             (W @ ar_flat).astype(f32)], axis=1)
        common[f"WW{li}"] = np.ascontiguousarray(WW)
        common[f"br{li}"] = np.tile(b[None, :], (P, 1)).astype(f32)

    feats = np.asarray(inputs["features"], f32)
    in_maps = []
    for c in range(n_cores):
        pc = per_core[c]
        fo = np.zeros((ng * P, 128), f32)
        fo[pc["srow"]] = feats[c * npc:(c + 1) * npc]
        in_maps.append({
            **common,
            "features_own": fo,
            "idx_lo": pc["idx_lo"], "idx_hi": pc["idx_hi"],
            "oh": pc["oh"], "ohT": pc["ohT"],
        })
    return in_maps


def kernel(**inputs):
    from concourse import bass_utils

    src = np.asarray(inputs["src"]).astype(np.int64)
    dst = np.asarray(inputs["dst"]).astype(np.int64)
    n_nodes = np.asarray(inputs["features"]).shape[0]
    meta, per_core = _preprocess(src, dst, n_nodes, N_CORES)
    nc = _get_compiled(meta)
    in_maps = _make_in_maps(inputs, meta, per_core)
    n_cores = meta["n_cores"]
    res = bass_utils.run_bass_kernel_spmd(
        nc, in_maps, core_ids=list(range(n_cores)),
        trace=bool(last_run_info.get("trace", False)))
    last_run_info["exec_time_ns"] = res.exec_time_ns
    last_run_info["profile_json"] = res.profile_json
    last_run_info["res"] = res

    npc = meta["npc"]
    out = np.empty((n_nodes, OUT_DIM), np.float32)
    for c in range(n_cores):
        rows = per_core[c]["srow"]
        out[c * npc:(c + 1) * npc] = res.results[c]["out"][rows]
    return out


# revision 8
# speedup vs baseline: 1.6891x; 1.0516x over previous
"""3-layer GAT (graph attention network) on Trainium2 — Bass/Tile, 8-core SPMD.

Sharding: nodes are partitioned into 8 contiguous ranges (graph/data
parallel).  Each core owns the edges whose *destination* falls in its range.

All per-core node data lives in "slot" order: destination nodes are packed
into ng groups of 128 slots (group g covers a window of <=128 consecutive
nodes); slot = g*128 + (node - group_base).  The host permutes the input
features into slot order, the epilogue writes outputs in slot order (so the
next layer's phase A needs NO gather), and per-group attention tables live
at static slot addresses (shared SPMD program across cores).

Per layer:
  phase A : one PE matmul per 128-slot block against [W | Wal | War] gives
            feat, el, er.  Table rows [feat|el] (bf16, 512 B) are written to
            DRAM; er (8 B/slot) to a packed local table.
  AllGather the table so every core can gather arbitrary src rows.
  edge    : per chunk, feat[src] rows are dma_gathered with indices split
            over all 4 SWDGE queues (desc-gen runs on a DSP pair per queue).
            One-hot matrices (host-built fp8, streamed from DRAM — both
            normal and transposed layouts) drive PE matmuls:
              er_edge = ohT.T @ er_blk            (per tile, [128,H])
              [msg | den] = oh.T @ [w*feat | w]   (accumulated per group)
            w = exp(leaky_relu(el+er)) via one fused DVE op + scalar Exp.
            Epilogue divides by den (edge softmax), adds bias, relu.
Edge softmax skips the segment-max subtraction: alpha = exp(e)/sum(exp(e))
is mathematically identical and the logits here are O(1).

dma_gather uses int16 indices (max 32767), so edges are split per group into
a "lo" zone (src slot < 32768) and a "hi" zone (gathered from a base-offset
view of the table).  Edges are sorted by src within each (group, zone) for
HBM locality.  All indices are valid (pads point at row 0; pad one-hot
columns are all-zero), so descriptor counts are compile-time constants.
"""

import numpy as np

try:
    import ml_dtypes
    _BF16 = ml_dtypes.bfloat16
    _FP8 = ml_dtypes.float8_e4m3
except ImportError:  # pragma: no cover
    _BF16 = None
    _FP8 = None

# ---------------- static problem config (self-contained) ---------------------
N_CORES = 8
NEG_SLOPE = 0.2
P = 128
GROUP_E = 2048             # max edges per PSUM accumulation group
CHUNK_GROUPS = 2           # groups per gather chunk
SPLIT = 32768              # int16 index split point
# (in_dim, H, D, apply_relu) per layer
LAYERS = [(128, 4, 32, True), (128, 4, 32, True), (128, 1, 64, False)]
OUT_DIM = 64
ROWS_L = (256, 256, 128)   # table row, bf16 elems: [feat | el | pad]
HMAX = 4                   # er table row width (elems)

_cache = {}
last_run_info = {}


# ============================ host-side preprocessing ========================

def _wrap16(vals, cols):
    """dma_gather index layout: entry i -> [i % 16, i // 16], replicated
    across the 8 groups of 16 partitions."""
    t = np.zeros((16, cols), np.int16)
    n = len(vals)
    t[np.arange(n) % 16, np.arange(n) // 16] = vals.astype(np.int16)
    return np.tile(t, (8, 1))


def _preprocess(src, dst, n_nodes, n_cores):
    npc = n_nodes // n_cores
    cores = []
    for c in range(n_cores):
        lo = c * npc
        m = (dst >= lo) & (dst < lo + npc)
        s = src[m].astype(np.int64)
        d = (dst[m] - lo).astype(np.int64)
        o = np.argsort(d, kind="stable")
        s, d = s[o], d[o]
        counts = np.bincount(d, minlength=npc)
        cum = np.zeros(npc + 1, np.int64)
        np.cumsum(counts, out=cum[1:])
        groups = []
        base = 0
        while base < npc:
            dmax = min(base + P, npc)
            limit = cum[base] + GROUP_E
            dend = int(np.searchsorted(cum, limit, side="right")) - 1
            dend = min(dend, dmax)
            if dend <= base:
                raise ValueError(f"dst {base} has degree > {GROUP_E}")
            groups.append((base, int(cum[base]), int(cum[dend])))
            base = dend
        cores.append((s, d, groups))

    ng = max(len(g) for (_, _, g) in cores)
    ng = ((ng + CHUNK_GROUPS - 1) // CHUNK_GROUPS) * CHUNK_GROUPS
    SLOTS = ng * P                       # per-core slot rows
    # node -> slot map per core (slot = gi*128 + node - base), -1 for none
    node_slot = []
    for c, (s, d, groups) in enumerate(cores):
        srow = np.zeros(npc, np.int64)
        for gi, (b, e0, e1) in enumerate(groups):
            b_next = groups[gi + 1][0] if gi + 1 < len(groups) else npc
            srow[b:b_next] = gi * P + (np.arange(b, b_next) - b)
        node_slot.append(srow)
    # global slot of node n
    gslot = np.empty(n_nodes, np.int64)
    for c in range(n_cores):
        gslot[c * npc:(c + 1) * npc] = c * SLOTS + node_slot[c]

    # per-group lo/hi tile counts (max across cores -> shared program)
    TL = np.zeros(ng, np.int64)
    TH = np.zeros(ng, np.int64)
    for (s, d, groups) in cores:
        gs = gslot[s]                    # src as global slot
        for gi, (b, e0, e1) in enumerate(groups):
            nlo = int((gs[e0:e1] < SPLIT).sum())
            nhi = (e1 - e0) - nlo
            TL[gi] = max(TL[gi], (nlo + P - 1) // P)
            TH[gi] = max(TH[gi], (nhi + P - 1) // P)
    lo_base = np.zeros(ng + 1, np.int64)
    hi_base = np.zeros(ng + 1, np.int64)
    np.cumsum(TL, out=lo_base[1:])
    np.cumsum(TH, out=hi_base[1:])
    SL = int(lo_base[ng]) * P
    SH = int(hi_base[ng]) * P
    SLP = max(SL, 2048)
    SHP = max(SH, 2048)
    CT = SL // P + SH // P               # total tiles, chunk-major layout

    per_core = []
    for c, (s, d, groups) in enumerate(cores):
        gs = gslot[s]
        idx_lo = np.zeros(SL, np.int64)
        idx_hi = np.zeros(SH, np.int64)
        # chunk-major dstrel per slot (-1 = pad) for host one-hot build
        dr_cm = np.full(CT * P, -1.0, np.float32)
        for gi, (b, e0, e1) in enumerate(groups):
            eg_s = gs[e0:e1]
            eg_d = d[e0:e1]
            lm = eg_s < SPLIT
            g0 = (gi // CHUNK_GROUPS) * CHUNK_GROUPS
            g1 = min(g0 + CHUNK_GROUPS, ng)
            cm0 = int(lo_base[g0] + hi_base[g0])
            ltc = int(lo_base[g1] - lo_base[g0])
            for zone, msk in ((0, lm), (1, ~lm)):
                zs = eg_s[msk]
                zd = eg_d[msk]
                o2 = np.argsort(zs, kind="stable")    # src-sorted for HBM
                zs, zd = zs[o2], zd[o2]
                n = len(zs)
                if zone == 0:
                    o = int(lo_base[gi]) * P
                    idx_lo[o:o + n] = zs
                    cmo = (cm0 + int(lo_base[gi] - lo_base[g0])) * P
                else:
                    o = int(hi_base[gi]) * P
                    idx_hi[o:o + n] = zs - SPLIT
                    cmo = (cm0 + ltc + int(hi_base[gi] - hi_base[g0])) * P
                dr_cm[cmo:cmo + n] = (zd - b).astype(np.float32)

        # one-hot (fp8) in chunk-major tile order; oh: [e%128, tile, j]
        # layout [P, CT*128]; ohT: [j, tile, e] layout [P, CT*128]
        drt = dr_cm.reshape(CT, P)                    # [tile, e]
        eye = (drt[:, :, None] == np.arange(P, dtype=np.float32)[None, None, :])
        oh = eye.astype(_FP8)                         # [tile, e, j]
        oh_d = np.ascontiguousarray(
            oh.transpose(1, 0, 2).reshape(P, CT * P))  # [e, tile*j]
        ohT_d = np.ascontiguousarray(
            oh.transpose(2, 0, 1).reshape(P, CT * P))  # [j, tile*e]

        def _padcols(a, cols):
            out = np.zeros((a.shape[0], cols), a.dtype)
            out[:, :a.shape[1]] = a
            return out

        per_core.append(dict(
            idx_lo=_padcols(_wrap16(idx_lo, max(SL // 16, 1)), SLP // 16),
            idx_hi=_padcols(_wrap16(idx_hi, max(SH // 16, 1)), SHP // 16),
            oh=oh_d, ohT=ohT_d,
            srow=node_slot[c],
        ))
    meta = dict(ng=ng, TL=tuple(int(x) for x in TL),
                TH=tuple(int(x) for x in TH), SL=SL, SH=SH,
                SLP=SLP, SHP=SHP, CT=CT,
                npc=npc, n_nodes=n_nodes, n_cores=n_cores)
    return meta, per_core


# ============================ device program =================================

def _build_program(meta):
    import concourse.bass as bass
    import concourse.tile as tile
    from concourse import bacc, mybir

    f32 = mybir.dt.float32
    bf16 = mybir.dt.bfloat16
    fp8 = mybir.dt.float8e4
    i16 = mybir.dt.int16
    AF = mybir.ActivationFunctionType
    OP = mybir.AluOpType

    ng, SL, SH = meta["ng"], meta["SL"], meta["SH"]
    SLP, SHP, CT = meta["SLP"], meta["SHP"], meta["CT"]
    TL, TH = meta["TL"], meta["TH"]
    npc = meta["npc"]
    n_cores = meta["n_cores"]
    SLOTS = ng * P
    NTOT = SLOTS * n_cores               # all-gathered table rows
    lo_base = np.concatenate([[0], np.cumsum(TL)]).astype(int)
    hi_base = np.concatenate([[0], np.cumsum(TH)]).astype(int)
    nchunk = ng // CHUNK_GROUPS

    nc = bacc.Bacc("TRN2", target_bir_lowering=False, debug=False,
                   enable_asserts=False, num_devices=n_cores,
                   num_swdge_queues=4)

    def _gather4(out_ap3, in_ap, idxs2, t0, t1, elem):
        """gather tiles [t0,t1) of a zone, striped over the 4 queues."""
        nt = t1 - t0
        step = (nt + 3) // 4
        q = 0
        for j0 in range(0, nt, step):
            j1 = min(j0 + step, nt)
            nc.gpsimd.dma_gather(
                out_ap=out_ap3[:, j0:j1, :],
                in_ap=in_ap,
                idxs_ap=idxs2[:, (t0 + j0) * 8:(t0 + j1) * 8],
                num_idxs=(j1 - j0) * P,
                num_idxs_reg=(j1 - j0) * P,
                elem_size=elem,
                single_packet=False,
                queue_num=q,
            )
            q += 1

    t_feats = nc.dram_tensor("features_own", [SLOTS, 128], f32,
                             kind="ExternalInput").ap()
    t_idx_lo = nc.dram_tensor("idx_lo", [P, SLP // 16], i16,
                              kind="ExternalInput").ap()
    t_idx_hi = nc.dram_tensor("idx_hi", [P, SHP // 16], i16,
                              kind="ExternalInput").ap()
    t_oh = nc.dram_tensor("oh", [P, CT * P], fp8, kind="ExternalInput").ap()
    t_ohT = nc.dram_tensor("ohT", [P, CT * P], fp8, kind="ExternalInput").ap()
    t_ident = nc.dram_tensor("identity", [P, P], f32,
                             kind="ExternalInput").ap()
    t_WW, t_b = [], []
    for li, (ind, H, D, _) in enumerate(LAYERS):
        hd = H * D
        t_WW.append(nc.dram_tensor(f"WW{li}", [ind, hd + 2 * H], f32,
                                   kind="ExternalInput").ap())
        t_b.append(nc.dram_tensor(f"br{li}", [P, hd], f32,
                                  kind="ExternalInput").ap())
    t_out = nc.dram_tensor("out", [SLOTS, OUT_DIM], f32,
                           kind="ExternalOutput").ap()

    with tile.TileContext(nc) as tc:
        with (
            tc.tile_pool(name="const", bufs=1) as cpool,
            tc.tile_pool(name="big", bufs=1) as bigpool,
            tc.tile_pool(name="sb", bufs=3) as sb,
            tc.tile_pool(name="fg", bufs=2) as fgpool,
            tc.tile_pool(name="wp", bufs=3) as wpool,
            tc.tile_pool(name="ps", bufs=3, space="PSUM") as pspool,
            tc.tile_pool(name="per", bufs=2, space="PSUM") as perpool,
            tc.tile_pool(name="psA", bufs=2, space="PSUM") as psA,
            tc.tile_pool(name="psB", bufs=1, space="PSUM") as psB,
            tc.tile_pool(name="dram", bufs=1, space="DRAM") as dram,
        ):
            # ---- constants ----
            ident = cpool.tile([P, P], f32)
            nc.sync.dma_start(ident[:], t_ident)
            idx_lo = cpool.tile([P, SLP // 16], i16)
            nc.sync.dma_start(idx_lo[:], t_idx_lo)
            idx_hi = cpool.tile([P, SHP // 16], i16)
            nc.sync.dma_start(idx_hi[:], t_idx_hi)
            WWs, Bs = [], []
            for li, (ind, H, D, _) in enumerate(LAYERS):
                hd = H * D
                w = cpool.tile([ind, hd + 2 * H], f32, tag=f"WW{li}")
                nc.sync.dma_start(w[:], t_WW[li])
                WWs.append(w)
                bb = cpool.tile([P, hd], f32, tag=f"br{li}")
                nc.sync.dma_start(bb[:], t_b[li])
                Bs.append(bb)

            prev_scratch = None
            for li, (ind, H, D, apply_relu) in enumerate(LAYERS):
                hd = H * D
                ROW = ROWS_L[li]
                # ---------------- phase A ----------------
                x_own = bigpool.tile([P, ng * ind], f32, tag="x_own")
                if li == 0:
                    nc.sync.dma_start(
                        x_own[:].rearrange("p (i d) -> p i d", d=ind),
                        t_feats.rearrange("(i p) d -> p i d", p=P))
                else:
                    nc.sync.dma_start(
                        x_own[:].rearrange("p (i d) -> p i d", d=ind),
                        prev_scratch[:].rearrange("(i p) d -> p i d", p=P))
                # table rows [feat | el | junk]
                tabsb = bigpool.tile([P, ng * ROW], bf16, tag="tabsb")
                er_own = sb.tile([P, ng * HMAX], bf16, tag="er_own")
                for i in range(ng):
                    xT_ps = psA.tile([P, P], f32, tag="psA")
                    nc.tensor.transpose(
                        out=xT_ps[:], in_=x_own[:, i * ind:(i + 1) * ind],
                        identity=ident[:])
                    xT = sb.tile([P, ind], f32, tag="xT")
                    nc.any.tensor_copy(xT[:], xT_ps[:, :ind])
                    f_ps = psB.tile([P, hd + 2 * H], f32, tag="psB")
                    nc.tensor.matmul(out=f_ps[:], lhsT=xT[:],
                                     rhs=WWs[li][:], start=True, stop=True)
                    nc.any.tensor_copy(
                        tabsb[:, i * ROW:i * ROW + hd + H],
                        f_ps[:, :hd + H])
                    nc.any.tensor_copy(er_own[:, i * HMAX:i * HMAX + H],
                                       f_ps[:, hd + H:hd + 2 * H])

                tab_own_d = dram.tile([SLOTS, ROW], bf16, tag=f"tab_own{li}")
                nc.sync.dma_start(
                    tab_own_d[:].rearrange("(i p) d -> p i d", p=P),
                    tabsb[:].rearrange("p (d2 d) -> p d2 d", d=ROW))
                er_own_d = dram.tile([SLOTS, HMAX], bf16, tag=f"er_own{li}")
                nc.sync.dma_start(
                    er_own_d[:].rearrange("(i p) h -> p i h", p=P),
                    er_own[:].rearrange("p (i h) -> p i h", h=HMAX))

                # ---------------- all-gather ----------------
                tab_full = dram.tile([NTOT, ROW], bf16,
                                     addr_space="Shared", tag=f"tab_full{li}")
                if n_cores == 1:
                    nc.sync.dma_start(tab_full[:], tab_own_d[:])
                else:
                    nc.gpsimd.collective_compute(
                        "AllGather", mybir.AluOpType.bypass,
                        replica_groups=[list(range(n_cores))],
                        ins=[tab_own_d[:]],
                        outs=[tab_full[:]],
                    )

                # ---------------- edge phase ----------------
                if li < 2:
                    scratch = dram.tile([SLOTS, hd], f32, tag=f"scratch{li}")
                else:
                    scratch = None

                for k in range(nchunk):
                    g0 = k * CHUNK_GROUPS
                    g1 = g0 + CHUNK_GROUPS
                    lt0, lt1 = int(lo_base[g0]), int(lo_base[g1])
                    ht0, ht1 = int(hi_base[g0]), int(hi_base[g1])
                    ltc, htc = lt1 - lt0, ht1 - ht0
                    ct = ltc + htc
                    cm0 = lt0 + ht0
                    if ct == 0:
                        for g in range(g0, g1):
                            dst_ap = (scratch[g * P:(g + 1) * P, :]
                                      if scratch is not None
                                      else t_out[g * P:(g + 1) * P, :])
                            ot = sb.tile([P, hd], f32, tag="ot")
                            if apply_relu:
                                nc.vector.tensor_scalar_max(
                                    ot[:], Bs[li][:, :hd], 0.0)
                            else:
                                nc.vector.tensor_copy(ot[:], Bs[li][:, :hd])
                            nc.sync.dma_start(dst_ap, ot[:])
                        continue
                    # one-hot (normal + transposed) streamed from DRAM
                    oh = fgpool.tile([P, ct * P], fp8, tag="oh")
                    nc.sync.dma_start(oh[:], t_oh[:, cm0 * P:(cm0 + ct) * P])
                    ohT = fgpool.tile([P, ct * P], fp8, tag="ohT")
                    nc.sync.dma_start(ohT[:],
                                      t_ohT[:, cm0 * P:(cm0 + ct) * P])
                    # er for the chunk's groups (static slot addresses)
                    er_blks = {}
                    for g in range(g0, g1):
                        eb = sb.tile([P, HMAX], bf16, tag="er_blk")
                        nc.sync.dma_start(eb[:],
                                          er_own_d[g * P:(g + 1) * P, :])
                        er_blks[g] = eb
                    # feat gathers, striped over the 4 queues
                    zones = {}
                    if ltc:
                        fgt = fgpool.tile([P, ltc * ROW], bf16, tag="fg_lo")
                        _gather4(fgt[:].rearrange("p (j d) -> p j d", d=ROW),
                                 tab_full[:SPLIT, :], idx_lo, lt0, lt1, ROW)
                        zones["lo"] = fgt
                    if htc:
                        fgt = fgpool.tile([P, htc * ROW], bf16, tag="fg_hi")
                        _gather4(fgt[:].rearrange("p (j d) -> p j d", d=ROW),
                                 tab_full[SPLIT:, :], idx_hi, ht0, ht1, ROW)
                        zones["hi"] = fgt

                    # chunk-tile index ci -> group, for both zones
                    tiles_of = {g: (list(range(int(lo_base[g]) - lt0,
                                               int(lo_base[g + 1]) - lt0)) +
                                    list(range(ltc + int(hi_base[g]) - ht0,
                                               ltc + int(hi_base[g + 1])
                                               - ht0)))
                                for g in range(g0, g1)}

                    # er_edge per tile via PE: ohT_t.T @ er_blk (all hoisted)
                    er_ps = perpool.tile([P, ct * H], f32, tag="er")
                    for g in range(g0, g1):
                        for ci in tiles_of[g]:
                            nc.tensor.matmul(
                                out=er_ps[:, ci * H:(ci + 1) * H],
                                lhsT=ohT[:, ci * P:(ci + 1) * P],
                                rhs=er_blks[g][:, :H],
                                start=True, stop=True)
                    # esum = el + er ; w = exp(lrelu)  (whole chunk at once)
                    esum = wpool.tile([P, ct * H], f32, tag="esum")
                    for (zoff, znt, zn) in ((0, ltc, "lo"), (ltc, htc, "hi")):
                        if znt == 0:
                            continue
                        nc.vector.tensor_tensor(
                            out=esum[:, zoff * H:(zoff + znt) * H]
                                .rearrange("p (t h) -> p t h", h=H),
                            in0=zones[zn][:]
                                .rearrange("p (t d) -> p t d",
                                           d=ROW)[:, :, hd:hd + H],
                            in1=er_ps[:, zoff * H:(zoff + znt) * H]
                                .rearrange("p (t h) -> p t h", h=H),
                            op=OP.add)
                    lrl = wpool.tile([P, ct * H], f32, tag="lrl")
                    nc.vector.scalar_tensor_tensor(
                        out=lrl[:], in0=esum[:], scalar=NEG_SLOPE,
                        in1=esum[:], op0=OP.mult, op1=OP.max)
                    wch = wpool.tile([P, ct * H], bf16, tag="w")
                    nc.scalar.activation(wch[:], lrl[:], AF.Exp)
                    # mge = [w*feat | w]  (whole chunk at once)
                    mge = sb.tile([P, ct * (hd + H)], bf16, tag="mge")
                    for (zoff, znt, zn) in ((0, ltc, "lo"), (ltc, htc, "hi")):
                        if znt == 0:
                            continue
                        nc.vector.tensor_tensor(
                            out=mge[:, zoff * (hd + H):
                                    (zoff + znt) * (hd + H)]
                                .rearrange("p (t e) -> p t e",
                                           e=hd + H)[:, :, :hd]
                                .rearrange("p t (h d) -> p t h d", d=D),
                            in0=zones[zn][:]
                                .rearrange("p (t d) -> p t d",
                                           d=ROW)[:, :, :hd]
                                .rearrange("p t (h d) -> p t h d", d=D),
                            in1=wch[:, zoff * H:(zoff + znt) * H]
                                .rearrange("p (t h) -> p t h", h=H)
                                .to_broadcast([P, znt, H, D]),
                            op=OP.mult)
                    nc.any.tensor_copy(
                        mge[:].rearrange("p (t e) -> p t e",
                                         e=hd + H)[:, :, hd:],
                        wch[:].rearrange("p (t h) -> p t h", h=H))

                    for g in range(g0, g1):
                        tg = tiles_of[g]
                        dst_ap = (scratch[g * P:(g + 1) * P, :]
                                  if scratch is not None
                                  else t_out[g * P:(g + 1) * P, :])
                        if not tg:
                            ot = sb.tile([P, hd], f32, tag="ot")
                            if apply_relu:
                                nc.vector.tensor_scalar_max(
                                    ot[:], Bs[li][:, :hd], 0.0)
                            else:
                                nc.vector.tensor_copy(ot[:], Bs[li][:, :hd])
                            nc.sync.dma_start(dst_ap, ot[:])
                            continue
                        # segment matmuls, denominator folded
                        ps = pspool.tile([P, hd + H], f32, tag="ps")
                        for n, ci in enumerate(tg):
                            nc.tensor.matmul(
                                out=ps[:],
                                lhsT=oh[:, ci * P:(ci + 1) * P],
                                rhs=mge[:, ci * (hd + H):
                                        (ci + 1) * (hd + H)],
                                start=(n == 0),
                                stop=(n == len(tg) - 1))
                        # epilogue: divide, bias, relu
                        den = sb.tile([P, H], f32, tag="den")
                        nc.vector.tensor_scalar_max(den[:], ps[:, hd:hd + H],
                                                    1e-12)
                        rec = sb.tile([P, H], f32, tag="rec")
                        nc.vector.reciprocal(rec[:], den[:])
                        ot = sb.tile([P, hd], f32, tag="ot")
                        nc.vector.tensor_tensor(
                            out=ot[:].rearrange("p (h d) -> p h d", d=D),
                            in0=ps[:, :hd].rearrange("p (h d) -> p h d", d=D),
                            in1=rec[:].to_broadcast([P, H, D]),
                            op=OP.mult)
                        nc.vector.tensor_tensor(out=ot[:], in0=ot[:],
                                                in1=Bs[li][:, :hd], op=OP.add)
                        if apply_relu:
                            nc.vector.tensor_scalar_max(ot[:], ot[:], 0.0)
                        nc.sync.dma_start(dst_ap, ot[:])
                prev_scratch = scratch
    nc.compile()
    return nc


# ============================ entry point ====================================

def _meta_key(meta):
    return (meta["ng"], meta["TL"], meta["TH"], meta["SL"], meta["SH"],
            meta["npc"], meta["n_nodes"], meta["n_cores"])


def _get_compiled(meta):
    key = _meta_key(meta)
    if key not in _cache:
        _cache[key] = _build_program(meta)
    return _cache[key]


def _make_in_maps(inputs, meta, per_core):
    f32 = np.float32
    npc = meta["npc"]
    ng = meta["ng"]
    n_cores = meta["n_cores"]
    ident = np.eye(P, dtype=f32)
    common = {"identity": ident}
    for li in range(len(LAYERS)):
        ind, H, D, _ = LAYERS[li]
        W = np.asarray(inputs[f"W{li}"], f32)
        al = np.asarray(inputs[f"al{li}"], f32)
        ar = np.asarray(inputs[f"ar{li}"], f32)
        b = np.asarray(inputs[f"b{li}"], f32)
        hd = H * D
        al_flat = np.zeros((hd, H), f32)
        ar_flat = np.zeros((hd, H), f32)
        for h in range(H):
            al_flat[h * D:(h + 1) * D, h] = al[h]
            ar_flat[h * D:(h + 1) * D, h] = ar[h]
        WW = np.concatenate([W, (W @ al_flat).astype(f32),
                             (W @ ar_flat).astype(f32)], axis=1)
        common[f"WW{li}"] = np.ascontiguousarray(WW)
        common[f"br{li}"] = np.tile(b[None, :], (P, 1)).astype(f32)

    feats = np.asarray(inputs["features"], f32)
    in_maps = []
    for c in range(n_cores):
        pc = per_core[c]
        fo = np.zeros((ng * P, 128), f32)
        fo[pc["srow"]] = feats[c * npc:(c + 1) * npc]
        in_maps.append({
            **common,
            "features_own": fo,
            "idx_lo": pc["idx_lo"], "idx_hi": pc["idx_hi"],
            "oh": pc["oh"], "ohT": pc["ohT"],
        })
    return in_maps


def kernel(**inputs):
    from concourse import bass_utils

    src = np.asarray(inputs["src"]).astype(np.int64)
    dst = np.asarray(inputs["dst"]).astype(np.int64)
    n_nodes = np.asarray(inputs["features"]).shape[0]
    meta, per_core = _preprocess(src, dst, n_nodes, N_CORES)
    nc = _get_compiled(meta)
    in_maps = _make_in_maps(inputs, meta, per_core)
    n_cores = meta["n_cores"]
    res = bass_utils.run_bass_kernel_spmd(
        nc, in_maps, core_ids=list(range(n_cores)),
        trace=bool(last_run_info.get("trace", False)))
    last_run_info["exec_time_ns"] = res.exec_time_ns
    last_run_info["profile_json"] = res.profile_json
    last_run_info["res"] = res

    npc = meta["npc"]
    out = np.empty((n_nodes, OUT_DIM), np.float32)
    for c in range(n_cores):
        rows = per_core[c]["srow"]
        out[c * npc:(c + 1) * npc] = res.results[c]["out"][rows]
    return out


# revision 9
# speedup vs baseline: 1.9300x; 1.1426x over previous
"""3-layer GAT (graph attention network) on Trainium2 — Bass/Tile, 8-core SPMD.

Sharding: nodes are partitioned into 8 contiguous ranges (graph/data
parallel).  Each core owns the edges whose *destination* falls in its range.

All per-core node data lives in "slot" order: destination nodes are packed
into ng groups of 128 slots (group g covers a window of <=128 consecutive
nodes); slot = g*128 + (node - group_base).  The host permutes the input
features into slot order, the epilogue writes outputs in slot order (so the
next layer's phase A needs NO gather), and per-group attention tables live
at static slot addresses (shared SPMD program across cores).

Per layer:
  phase A : one PE matmul per 128-slot block against [W | War] gives feat
            and er.  Feat rows (bf16, 256 B) form the gather table (written
            in two halves so the first AllGather fires at phase-A midpoint);
            er (8 B/slot) goes to a packed local table.
  2x AllGather (half tables) so every core can gather arbitrary src rows.
  edge    : per chunk, feat[src] rows are dma_gathered with indices split
            over all 4 SWDGE queues (desc-gen runs on a DSP pair per queue).
            One-hot matrices (host-built fp8, streamed from DRAM — both
            normal and transposed layouts) drive PE matmuls:
              er_edge = ohT_t.T @ er_blk          (per tile, [128,H])
              [msg | den] = oh_t.T @ [w*feat | w] (accumulated per group)
            el is computed on device (feat*al, reduce over D); w =
            exp(leaky_relu(el+er)) via one fused DVE op + scalar Exp.
            Epilogue divides by den (edge softmax), adds bias, relu.
Edge softmax skips the segment-max subtraction: alpha = exp(e)/sum(exp(e))
is mathematically identical and the logits here are O(1).

dma_gather uses int16 indices (max 32767): edges are split per group into
zone "lo" (src in the first half of its core's slots, gathered from the
half-A table) and zone "hi" (second half, half-B table); 8*SLOTS/2 = 26624
rows per half table, so all indices fit int16 with no offset views.
Edges are sorted by src within each (group, zone) for HBM locality.
"""

import numpy as np

try:
    import ml_dtypes
    _BF16 = ml_dtypes.bfloat16
    _FP8 = ml_dtypes.float8_e4m3
except ImportError:  # pragma: no cover
    _BF16 = None
    _FP8 = None

# ---------------- static problem config (self-contained) ---------------------
N_CORES = 8
NEG_SLOPE = 0.2
P = 128
GROUP_E = 2048             # max edges per PSUM accumulation group
CHUNK_GROUPS = 2           # groups per gather chunk
# (in_dim, H, D, apply_relu) per layer
LAYERS = [(128, 4, 32, True), (128, 4, 32, True), (128, 1, 64, False)]
OUT_DIM = 64
ROW = 128                  # table row, bf16 elems (256 B): [feat | pad]
HMAX = 4                   # er table row width (elems)

_cache = {}
last_run_info = {}


# ============================ host-side preprocessing ========================

def _wrap16(vals, cols):
    """dma_gather index layout: entry i -> [i % 16, i // 16], replicated
    across the 8 groups of 16 partitions."""
    t = np.zeros((16, cols), np.int16)
    n = len(vals)
    t[np.arange(n) % 16, np.arange(n) // 16] = vals.astype(np.int16)
    return np.tile(t, (8, 1))


def _preprocess(src, dst, n_nodes, n_cores):
    npc = n_nodes // n_cores
    cores = []
    for c in range(n_cores):
        lo = c * npc
        m = (dst >= lo) & (dst < lo + npc)
        s = src[m].astype(np.int64)
        d = (dst[m] - lo).astype(np.int64)
        o = np.argsort(d, kind="stable")
        s, d = s[o], d[o]
        counts = np.bincount(d, minlength=npc)
        cum = np.zeros(npc + 1, np.int64)
        np.cumsum(counts, out=cum[1:])
        groups = []
        base = 0
        while base < npc:
            dmax = min(base + P, npc)
            limit = cum[base] + GROUP_E
            dend = int(np.searchsorted(cum, limit, side="right")) - 1
            dend = min(dend, dmax)
            if dend <= base:
                raise ValueError(f"dst {base} has degree > {GROUP_E}")
            groups.append((base, int(cum[base]), int(cum[dend])))
            base = dend
        cores.append((s, d, groups))

    ng = max(len(g) for (_, _, g) in cores)
    ng = ((ng + CHUNK_GROUPS - 1) // CHUNK_GROUPS) * CHUNK_GROUPS
    if ng % 2:
        ng += ng % CHUNK_GROUPS or CHUNK_GROUPS  # keep even & chunk-multiple
    SLOTS = ng * P
    ng2 = ng // 2
    S2 = ng2 * P                          # half-table rows per core
    # node -> slot map per core
    node_slot = []
    for c, (s, d, groups) in enumerate(cores):
        srow = np.zeros(npc, np.int64)
        for gi, (b, e0, e1) in enumerate(groups):
            b_next = groups[gi + 1][0] if gi + 1 < len(groups) else npc
            srow[b:b_next] = gi * P + (np.arange(b, b_next) - b)
        node_slot.append(srow)
    # zone (0 = first half of owner core's slots) and half-table index
    zidx = np.empty(n_nodes, np.int64)
    zzone = np.empty(n_nodes, np.bool_)
    for c in range(n_cores):
        sl = node_slot[c]
        hi = sl >= S2
        zzone[c * npc:(c + 1) * npc] = hi
        zi = np.where(hi, c * S2 + (sl - S2), c * S2 + sl)
        zidx[c * npc:(c + 1) * npc] = zi
    assert n_cores * S2 <= 32768

    TL = np.zeros(ng, np.int64)
    TH = np.zeros(ng, np.int64)
    for (s, d, groups) in cores:
        hz = zzone[s]
        for gi, (b, e0, e1) in enumerate(groups):
            nhi = int(hz[e0:e1].sum())
            nlo = (e1 - e0) - nhi
            TL[gi] = max(TL[gi], (nlo + P - 1) // P)
            TH[gi] = max(TH[gi], (nhi + P - 1) // P)
    lo_base = np.zeros(ng + 1, np.int64)
    hi_base = np.zeros(ng + 1, np.int64)
    np.cumsum(TL, out=lo_base[1:])
    np.cumsum(TH, out=hi_base[1:])
    SL = int(lo_base[ng]) * P
    SH = int(hi_base[ng]) * P
    SLP = max(SL, 2048)
    SHP = max(SH, 2048)
    CT = SL // P + SH // P               # total tiles, chunk-major layout

    per_core = []
    for c, (s, d, groups) in enumerate(cores):
        gz = zzone[s]
        gi_idx = zidx[s]
        idx_lo = np.zeros(SL, np.int64)
        idx_hi = np.zeros(SH, np.int64)
        dr_cm = np.full(CT * P, -1.0, np.float32)
        for gi, (b, e0, e1) in enumerate(groups):
            eg_i = gi_idx[e0:e1]
            eg_d = d[e0:e1]
            hm = gz[e0:e1]
            g0 = (gi // CHUNK_GROUPS) * CHUNK_GROUPS
            g1 = min(g0 + CHUNK_GROUPS, ng)
            cm0 = int(lo_base[g0] + hi_base[g0])
            ltc = int(lo_base[g1] - lo_base[g0])
            for zone, msk in ((0, ~hm), (1, hm)):
                zs = eg_i[msk]
                zd = eg_d[msk]
                o2 = np.argsort(zs, kind="stable")    # src-sorted for HBM
                zs, zd = zs[o2], zd[o2]
                n = len(zs)
                if zone == 0:
                    o = int(lo_base[gi]) * P
                    idx_lo[o:o + n] = zs
                    cmo = (cm0 + int(lo_base[gi] - lo_base[g0])) * P
                else:
                    o = int(hi_base[gi]) * P
                    idx_hi[o:o + n] = zs
                    cmo = (cm0 + ltc + int(hi_base[gi] - hi_base[g0])) * P
                dr_cm[cmo:cmo + n] = (zd - b).astype(np.float32)

        drt = dr_cm.reshape(CT, P)                    # [tile, e]
        eye = (drt[:, :, None] ==
               np.arange(P, dtype=np.float32)[None, None, :])
        oh = eye.astype(_FP8)                         # [tile, e, j]
        oh_d = np.ascontiguousarray(
            oh.transpose(1, 0, 2).reshape(P, CT * P))  # [e, tile*j]
        ohT_d = np.ascontiguousarray(
            oh.transpose(2, 0, 1).reshape(P, CT * P))  # [j, tile*e]

        def _padcols(a, cols):
            out = np.zeros((a.shape[0], cols), a.dtype)
            out[:, :a.shape[1]] = a
            return out

        per_core.append(dict(
            idx_lo=_padcols(_wrap16(idx_lo, max(SL // 16, 1)), SLP // 16),
            idx_hi=_padcols(_wrap16(idx_hi, max(SH // 16, 1)), SHP // 16),
            oh=oh_d, ohT=ohT_d,
            srow=node_slot[c],
        ))
    meta = dict(ng=ng, TL=tuple(int(x) for x in TL),
                TH=tuple(int(x) for x in TH), SL=SL, SH=SH,
                SLP=SLP, SHP=SHP, CT=CT,
                npc=npc, n_nodes=n_nodes, n_cores=n_cores)
    return meta, per_core


# ============================ device program =================================

def _build_program(meta):
    import concourse.bass as bass
    import concourse.tile as tile
    from concourse import bacc, mybir

    def _midb(ap, n):
        # [P, D] -> [P, n, D] with the middle dim broadcast (step 0)
        return bass.AP(ap.tensor, ap.offset,
                       [list(ap.ap[0]), [0, n], list(ap.ap[1])])

    f32 = mybir.dt.float32
    bf16 = mybir.dt.bfloat16
    fp8 = mybir.dt.float8e4
    i16 = mybir.dt.int16
    AF = mybir.ActivationFunctionType
    OP = mybir.AluOpType

    ng, SL, SH = meta["ng"], meta["SL"], meta["SH"]
    SLP, SHP, CT = meta["SLP"], meta["SHP"], meta["CT"]
    TL, TH = meta["TL"], meta["TH"]
    npc = meta["npc"]
    n_cores = meta["n_cores"]
    SLOTS = ng * P
    ng2 = ng // 2
    S2 = ng2 * P
    lo_base = np.concatenate([[0], np.cumsum(TL)]).astype(int)
    hi_base = np.concatenate([[0], np.cumsum(TH)]).astype(int)
    nchunk = ng // CHUNK_GROUPS

    nc = bacc.Bacc("TRN2", target_bir_lowering=False, debug=False,
                   enable_asserts=False, num_devices=n_cores,
                   num_swdge_queues=4)

    def _gather4(out_ap3, in_ap, idxs2, t0, t1, elem):
        """gather tiles [t0,t1) of a zone, striped over the 4 queues."""
        nt = t1 - t0
        step = (nt + 3) // 4
        q = 0
        for j0 in range(0, nt, step):
            j1 = min(j0 + step, nt)
            nc.gpsimd.dma_gather(
                out_ap=out_ap3[:, j0:j1, :],
                in_ap=in_ap,
                idxs_ap=idxs2[:, (t0 + j0) * 8:(t0 + j1) * 8],
                num_idxs=(j1 - j0) * P,
                num_idxs_reg=(j1 - j0) * P,
                elem_size=elem,
                single_packet=False,
                queue_num=q,
            )
            q += 1

    t_feats = nc.dram_tensor("features_own", [SLOTS, 128], f32,
                             kind="ExternalInput").ap()
    t_idx_lo = nc.dram_tensor("idx_lo", [P, SLP // 16], i16,
                              kind="ExternalInput").ap()
    t_idx_hi = nc.dram_tensor("idx_hi", [P, SHP // 16], i16,
                              kind="ExternalInput").ap()
    t_oh = nc.dram_tensor("oh", [P, CT * P], fp8, kind="ExternalInput").ap()
    t_ohT = nc.dram_tensor("ohT", [P, CT * P], fp8, kind="ExternalInput").ap()
    t_ident = nc.dram_tensor("identity", [P, P], f32,
                             kind="ExternalInput").ap()
    t_WW, t_b, t_al = [], [], []
    for li, (ind, H, D, _) in enumerate(LAYERS):
        hd = H * D
        t_WW.append(nc.dram_tensor(f"WW{li}", [ind, hd + H], f32,
                                   kind="ExternalInput").ap())
        t_b.append(nc.dram_tensor(f"br{li}", [P, hd], f32,
                                  kind="ExternalInput").ap())
        t_al.append(nc.dram_tensor(f"albc{li}", [P, hd], bf16,
                                   kind="ExternalInput").ap())
    t_out = nc.dram_tensor("out", [SLOTS, OUT_DIM], f32,
                           kind="ExternalOutput").ap()

    with tile.TileContext(nc) as tc:
        with (
            tc.tile_pool(name="const", bufs=1) as cpool,
            tc.tile_pool(name="big", bufs=1) as bigpool,
            tc.tile_pool(name="sb", bufs=3) as sb,
            tc.tile_pool(name="fg", bufs=2) as fgpool,
            tc.tile_pool(name="wp", bufs=2) as wpool,
            tc.tile_pool(name="ps", bufs=3, space="PSUM") as pspool,
            tc.tile_pool(name="per", bufs=2, space="PSUM") as perpool,
            tc.tile_pool(name="psA", bufs=2, space="PSUM") as psA,
            tc.tile_pool(name="psB", bufs=1, space="PSUM") as psB,
            tc.tile_pool(name="dram", bufs=1, space="DRAM") as dram,
        ):
            # ---- constants ----
            ident = cpool.tile([P, P], f32)
            nc.sync.dma_start(ident[:], t_ident)
            idx_lo = cpool.tile([P, SLP // 16], i16)
            nc.sync.dma_start(idx_lo[:], t_idx_lo)
            idx_hi = cpool.tile([P, SHP // 16], i16)
            nc.sync.dma_start(idx_hi[:], t_idx_hi)
            WWs, Bs, ALs = [], [], []
            for li, (ind, H, D, _) in enumerate(LAYERS):
                hd = H * D
                w = cpool.tile([ind, hd + H], f32, tag=f"WW{li}")
                nc.sync.dma_start(w[:], t_WW[li])
                WWs.append(w)
                bb = cpool.tile([P, hd], f32, tag=f"br{li}")
                nc.sync.dma_start(bb[:], t_b[li])
                Bs.append(bb)
                aa = cpool.tile([P, hd], bf16, tag=f"albc{li}")
                nc.sync.dma_start(aa[:], t_al[li])
                ALs.append(aa)

            prev_scratch = None
            for li, (ind, H, D, apply_relu) in enumerate(LAYERS):
                hd = H * D
                # ---------------- phase A (two halves) ----------------
                x_own = bigpool.tile([P, ng * ind], f32, tag="x_own")
                if li == 0:
                    nc.sync.dma_start(
                        x_own[:].rearrange("p (i d) -> p i d", d=ind),
                        t_feats.rearrange("(i p) d -> p i d", p=P))
                else:
                    nc.sync.dma_start(
                        x_own[:].rearrange("p (i d) -> p i d", d=ind),
                        prev_scratch[:].rearrange("(i p) d -> p i d", p=P))
                er_own_d = dram.tile([SLOTS, HMAX], bf16, tag=f"er_own{li}")
                halves = []
                for hf in range(2):
                    tabsb = bigpool.tile([P, ng2 * ROW], bf16,
                                         tag=f"tabsb{hf}")
                    er_own = sb.tile([P, ng2 * HMAX], bf16, tag=f"er_own{hf}")
                    for i2 in range(ng2):
                        i = hf * ng2 + i2
                        xT_ps = psA.tile([P, P], f32, tag="psA")
                        nc.tensor.transpose(
                            out=xT_ps[:], in_=x_own[:, i * ind:(i + 1) * ind],
                            identity=ident[:])
                        xT = sb.tile([P, ind], f32, tag="xT")
                        nc.vector.tensor_copy(xT[:], xT_ps[:, :ind])
                        f_ps = psB.tile([P, hd + H], f32, tag="psB")
                        nc.tensor.matmul(out=f_ps[:], lhsT=xT[:],
                                         rhs=WWs[li][:], start=True,
                                         stop=True)
                        nc.vector.tensor_copy(
                            tabsb[:, i2 * ROW:i2 * ROW + hd], f_ps[:, :hd])
                        nc.any.tensor_copy(
                            er_own[:, i2 * HMAX:i2 * HMAX + H],
                            f_ps[:, hd:hd + H])
                    tab_own = dram.tile([S2, ROW], bf16,
                                        tag=f"tab_own{li}_{hf}")
                    nc.sync.dma_start(
                        tab_own[:].rearrange("(i p) d -> p i d", p=P),
                        tabsb[:].rearrange("p (d2 d) -> p d2 d", d=ROW))
                    nc.sync.dma_start(
                        er_own_d[hf * S2:(hf + 1) * S2, :]
                        .rearrange("(i p) h -> p i h", p=P),
                        er_own[:].rearrange("p (i h) -> p i h", h=HMAX))
                    tab_full = dram.tile([n_cores * S2, ROW], bf16,
                                         addr_space="Shared",
                                         tag=f"tab_full{li}_{hf}")
                    if n_cores == 1:
                        nc.sync.dma_start(tab_full[:], tab_own[:])
                    else:
                        nc.gpsimd.collective_compute(
                            "AllGather", mybir.AluOpType.bypass,
                            replica_groups=[list(range(n_cores))],
                            ins=[tab_own[:]],
                            outs=[tab_full[:]],
                        )
                    halves.append(tab_full)

                # ---------------- edge phase ----------------
                if li < 2:
                    scratch = dram.tile([SLOTS, hd], f32, tag=f"scratch{li}")
                else:
                    scratch = None

                for k in range(nchunk):
                    g0 = k * CHUNK_GROUPS
                    g1 = g0 + CHUNK_GROUPS
                    lt0, lt1 = int(lo_base[g0]), int(lo_base[g1])
                    ht0, ht1 = int(hi_base[g0]), int(hi_base[g1])
                    ltc, htc = lt1 - lt0, ht1 - ht0
                    ct = ltc + htc
                    cm0 = lt0 + ht0
                    if ct == 0:
                        for g in range(g0, g1):
                            dst_ap = (scratch[g * P:(g + 1) * P, :]
                                      if scratch is not None
                                      else t_out[g * P:(g + 1) * P, :])
                            ot = sb.tile([P, hd], f32, tag="ot")
                            if apply_relu:
                                nc.vector.tensor_scalar_max(
                                    ot[:], Bs[li][:, :hd], 0.0)
                            else:
                                nc.vector.tensor_copy(ot[:], Bs[li][:, :hd])
                            nc.sync.dma_start(dst_ap, ot[:])
                        continue
                    # one-hot (normal + transposed) streamed from DRAM
                    oh = fgpool.tile([P, ct * P], fp8, tag="oh")
                    nc.sync.dma_start(oh[:], t_oh[:, cm0 * P:(cm0 + ct) * P])
                    ohT = fgpool.tile([P, ct * P], fp8, tag="ohT")
                    nc.sync.dma_start(ohT[:],
                                      t_ohT[:, cm0 * P:(cm0 + ct) * P])
                    # er for the chunk's groups (static slot addresses)
                    er_blks = {}
                    for g in range(g0, g1):
                        eb = sb.tile([P, HMAX], bf16, tag="er_blk")
                        nc.sync.dma_start(eb[:],
                                          er_own_d[g * P:(g + 1) * P, :])
                        er_blks[g] = eb
                    # feat gathers, striped over the 4 queues
                    zones = {}
                    if ltc:
                        fgt = fgpool.tile([P, ltc * ROW], bf16, tag="fg_lo")
                        _gather4(fgt[:].rearrange("p (j d) -> p j d", d=ROW),
                                 halves[0][:], idx_lo, lt0, lt1, ROW)
                        zones["lo"] = fgt
                    if htc:
                        fgt = fgpool.tile([P, htc * ROW], bf16, tag="fg_hi")
                        _gather4(fgt[:].rearrange("p (j d) -> p j d", d=ROW),
                                 halves[1][:], idx_hi, ht0, ht1, ROW)
                        zones["hi"] = fgt

                    tiles_of = {g: (list(range(int(lo_base[g]) - lt0,
                                               int(lo_base[g + 1]) - lt0)) +
                                    list(range(ltc + int(hi_base[g]) - ht0,
                                               ltc + int(hi_base[g + 1])
                                               - ht0)))
                                for g in range(g0, g1)}

                    # er_edge per tile via PE: ohT_t.T @ er_blk (all hoisted)
                    er_ps = perpool.tile([P, ct * H], f32, tag="er")
                    for g in range(g0, g1):
                        for ci in tiles_of[g]:
                            nc.tensor.matmul(
                                out=er_ps[:, ci * H:(ci + 1) * H],
                                lhsT=ohT[:, ci * P:(ci + 1) * P],
                                rhs=er_blks[g][:, :H],
                                start=True, stop=True)
                    # el = sum_d feat*al ; esum = el + er ; w = exp(lrelu)
                    esum = wpool.tile([P, ct * H], f32, tag="esum")
                    for (zoff, znt, zn) in ((0, ltc, "lo"), (ltc, htc, "hi")):
                        if znt == 0:
                            continue
                        prod = wpool.tile([P, znt * hd], bf16,
                                          tag=f"prod_{zn}")
                        nc.vector.tensor_tensor(
                            out=prod[:].rearrange("p (j d) -> p j d", d=hd),
                            in0=zones[zn][:].rearrange("p (j d) -> p j d",
                                                       d=ROW)[:, :, :hd],
                            in1=_midb(ALs[li][:], znt),
                            op=OP.mult)
                        nc.vector.tensor_reduce(
                            out=esum[:, zoff * H:(zoff + znt) * H],
                            in_=prod[:].rearrange("p (j h d) -> p (j h) d",
                                                  h=H, d=D),
                            axis=mybir.AxisListType.X, op=OP.add)
                    nc.vector.tensor_tensor(
                        out=esum[:], in0=esum[:], in1=er_ps[:],
                        op=OP.add)
                    lrl = wpool.tile([P, ct * H], f32, tag="lrl")
                    nc.vector.scalar_tensor_tensor(
                        out=lrl[:], in0=esum[:], scalar=NEG_SLOPE,
                        in1=esum[:], op0=OP.mult, op1=OP.max)
                    wch = wpool.tile([P, ct * H], bf16, tag="w")
                    nc.scalar.activation(wch[:], lrl[:], AF.Exp)
                    # mge = [w*feat | w]  (whole chunk at once)
                    mge = sb.tile([P, ct * (hd + H)], bf16, tag="mge")
                    for (zoff, znt, zn) in ((0, ltc, "lo"), (ltc, htc, "hi")):
                        if znt == 0:
                            continue
                        nc.vector.tensor_tensor(
                            out=mge[:, zoff * (hd + H):
                                    (zoff + znt) * (hd + H)]
                                .rearrange("p (t e) -> p t e",
                                           e=hd + H)[:, :, :hd]
                                .rearrange("p t (h d) -> p t h d", d=D),
                            in0=zones[zn][:]
                                .rearrange("p (t d) -> p t d",
                                           d=ROW)[:, :, :hd]
                                .rearrange("p t (h d) -> p t h d", d=D),
                            in1=wch[:, zoff * H:(zoff + znt) * H]
                                .rearrange("p (t h) -> p t h", h=H)
                                .to_broadcast([P, znt, H, D]),
                            op=OP.mult)
                    nc.any.tensor_copy(
                        mge[:].rearrange("p (t e) -> p t e",
                                         e=hd + H)[:, :, hd:],
                        wch[:].rearrange("p (t h) -> p t h", h=H))

                    for g in range(g0, g1):
                        tg = tiles_of[g]
                        dst_ap = (scratch[g * P:(g + 1) * P, :]
                                  if scratch is not None
                                  else t_out[g * P:(g + 1) * P, :])
                        if not tg:
                            ot = sb.tile([P, hd], f32, tag="ot")
                            if apply_relu:
                                nc.vector.tensor_scalar_max(
                                    ot[:], Bs[li][:, :hd], 0.0)
                            else:
                                nc.vector.tensor_copy(ot[:], Bs[li][:, :hd])
                            nc.sync.dma_start(dst_ap, ot[:])
                            continue
                        # segment matmuls, denominator folded
                        ps = pspool.tile([P, hd + H], f32, tag="ps")
                        for n, ci in enumerate(tg):
                            nc.tensor.matmul(
                                out=ps[:],
                                lhsT=oh[:, ci * P:(ci + 1) * P],
                                rhs=mge[:, ci * (hd + H):
                                        (ci + 1) * (hd + H)],
                                start=(n == 0),
                                stop=(n == len(tg) - 1))
                        # epilogue: divide, bias, relu
                        den = sb.tile([P, H], f32, tag="den")
                        nc.vector.tensor_scalar_max(den[:], ps[:, hd:hd + H],
                                                    1e-12)
                        rec = sb.tile([P, H], f32, tag="rec")
                        nc.vector.reciprocal(rec[:], den[:])
                        ot = sb.tile([P, hd], f32, tag="ot")
                        nc.vector.tensor_tensor(
                            out=ot[:].rearrange("p (h d) -> p h d", d=D),
                            in0=ps[:, :hd].rearrange("p (h d) -> p h d", d=D),
                            in1=rec[:].to_broadcast([P, H, D]),
                            op=OP.mult)
                        nc.vector.tensor_tensor(out=ot[:], in0=ot[:],
                                                in1=Bs[li][:, :hd], op=OP.add)
                        if apply_relu:
                            nc.vector.tensor_scalar_max(ot[:], ot[:], 0.0)
                        nc.sync.dma_start(dst_ap, ot[:])
                prev_scratch = scratch
    nc.compile()
    return nc


# ============================ entry point ====================================

def _meta_key(meta):
    return (meta["ng"], meta["TL"], meta["TH"], meta["SL"], meta["SH"],
            meta["npc"], meta["n_nodes"], meta["n_cores"])


def _get_compiled(meta):
    key = _meta_key(meta)
    if key not in _cache:
        _cache[key] = _build_program(meta)
    return _cache[key]


def _make_in_maps(inputs, meta, per_core):
    f32 = np.float32
    npc = meta["npc"]
    ng = meta["ng"]
    n_cores = meta["n_cores"]
    ident = np.eye(P, dtype=f32)
    common = {"identity": ident}
    for li in range(len(LAYERS)):
        ind, H, D, _ = LAYERS[li]
        W = np.asarray(inputs[f"W{li}"], f32)
        al = np.asarray(inputs[f"al{li}"], f32)
        ar = np.asarray(inputs[f"ar{li}"], f32)
        b = np.asarray(inputs[f"b{li}"], f32)
        hd = H * D
        ar_flat = np.zeros((hd, H), f32)
        for h in range(H):
            ar_flat[h * D:(h + 1) * D, h] = ar[h]
        WW = np.concatenate([W, (W @ ar_flat).astype(f32)], axis=1)
        common[f"WW{li}"] = np.ascontiguousarray(WW)
        common[f"br{li}"] = np.tile(b[None, :], (P, 1)).astype(f32)
        common[f"albc{li}"] = np.tile(al.reshape(1, hd), (P, 1)).astype(_BF16)

    feats = np.asarray(inputs["features"], f32)
    in_maps = []
    for c in range(n_cores):
        pc = per_core[c]
        fo = np.zeros((ng * P, 128), f32)
        fo[pc["srow"]] = feats[c * npc:(c + 1) * npc]
        in_maps.append({
            **common,
            "features_own": fo,
            "idx_lo": pc["idx_lo"], "idx_hi": pc["idx_hi"],
            "oh": pc["oh"], "ohT": pc["ohT"],
        })
    return in_maps


def kernel(**inputs):
    from concourse import bass_utils

    src = np.asarray(inputs["src"]).astype(np.int64)
    dst = np.asarray(inputs["dst"]).astype(np.int64)
    n_nodes = np.asarray(inputs["features"]).shape[0]
    meta, per_core = _preprocess(src, dst, n_nodes, N_CORES)
    nc = _get_compiled(meta)
    in_maps = _make_in_maps(inputs, meta, per_core)
    n_cores = meta["n_cores"]
    res = bass_utils.run_bass_kernel_spmd(
        nc, in_maps, core_ids=list(range(n_cores)),
        trace=bool(last_run_info.get("trace", False)))
    last_run_info["exec_time_ns"] = res.exec_time_ns
    last_run_info["profile_json"] = res.profile_json
    last_run_info["res"] = res

    npc = meta["npc"]
    out = np.empty((n_nodes, OUT_DIM), np.float32)
    for c in range(n_cores):
        rows = per_core[c]["srow"]
        out[c * npc:(c + 1) * npc] = res.results[c]["out"][rows]
    return out
